# revision 1
# baseline (speedup 1.0000x reference)
"""DeltaNet forward kernel for 8 Trainium2 NeuronCores.

Problem (hardcoded from the task spec): hidden_states [B=4, T=2048, D=1024],
H=4 heads, Dh=256, causal depthwise conv K=4 + silu on q/k/v projections,
q/k l2-normalized per head (q scaled Dh^-0.5), delta-rule recurrence over T,
per-head RMSNorm, merge heads, out = o @ Wo.

Sharding: data-parallel over (batch, head-group): core c -> batch c//2,
head group c%2 (projection columns [512*(c%2), 512*(c%2)+512)). Each core
computes a partial product against its 512 rows of Wo; the host sums the two
partials per batch (the unshard step for the row-parallel output matmul).

Device algorithm: chunked WY form of the delta rule, chunk C=128.
Per chunk (per head): KK = K K^T; A/B = strict lower/upper mask of KK;
R = (I+B)^{-1} = (I-B)(I+B^2)(I+B^4)...(I+B^64) via masked doubling
(B nilpotent); U' = R^T (V - K S); O = Q S + triu(K Q^T)^T U'; S += K^T U'.
S accumulates in PSUM in f32; matmul operands are bf16.
"""

import numpy as np

B, T, D = 4, 2048, 1024
H = 4
DH = D // H          # 256
CONV_K = 4
EPS = 1e-5
NCORES = 8
CG = 512             # columns per core (2 heads)
C = 128              # recurrence chunk length
NCHUNK = T // C      # 16
PAD = 4              # front zero padding on time axis for the causal conv
TOKB = 512           # token block (matmul moving size)
KT = D // 128        # 8 contraction tiles
CT = CG // 128       # 4 column tiles per core
NB = T // TOKB       # 4 token blocks

_CACHE = {}
SILU_NATIVE = True  # CoreSim lacks Silu; set False for simulation runs
DEBUG_SKIP_WO = False  # debug: skip output projection phase


def _build_bass():
    import concourse.bass as bass  # noqa: F401
    import concourse.bacc as bacc
    import concourse.mybir as mybir
    import concourse.tile as tile

    dt = mybir.dt
    nc = bacc.Bacc("TRN2", target_bir_lowering=False, debug=False)

    xT = nc.dram_tensor("xT", [D, T], dt.float16, kind="ExternalInput")
    wq = nc.dram_tensor("wq", [D, CG], dt.float16, kind="ExternalInput")
    wk = nc.dram_tensor("wk", [D, CG], dt.float16, kind="ExternalInput")
    wv = nc.dram_tensor("wv", [D, CG], dt.float16, kind="ExternalInput")
    wo = nc.dram_tensor("wo", [CG, D], dt.float16, kind="ExternalInput")
    cw = nc.dram_tensor("cw", [CG, 3 * CONV_K], dt.float32, kind="ExternalInput")
    consts = nc.dram_tensor("consts", [128, 6 * 128], dt.float16,
                            kind="ExternalInput")
    out = nc.dram_tensor("out", [T, D], dt.float32, kind="ExternalOutput")

    with tile.TileContext(nc) as tc:
        _body(nc, tc, mybir, xT, wq, wk, wv, wo, cw, consts, out)

    nc.compile()
    return nc


def _body(nc, tc, mybir, xT, wq, wk, wv, wo, cw, consts, out):
    dt = mybir.dt
    AF = mybir.ActivationFunctionType
    ALU = mybir.AluOpType
    fp32 = dt.float32
    bf16 = dt.float16  # 16-bit working dtype (fp16: 11-bit mantissa)
    NT = T + PAD

    xT_t = xT.ap().rearrange("(n p) t -> n p t", p=128)       # [8,128,T]
    w_t = {"q": wq.ap().rearrange("(n p) c -> n p c", p=128),
           "k": wk.ap().rearrange("(n p) c -> n p c", p=128),
           "v": wv.ap().rearrange("(n p) c -> n p c", p=128)}
    wo_t = wo.ap().rearrange("(n p) c -> n p c", p=128)       # [4,128,D]
    cw_t = cw.ap().rearrange("(n p) c -> n p c", p=128)       # [4,128,12]
    out_t = out.ap().rearrange("(n p) c -> n p c", p=128)     # [16,128,D]

    # ---------- persistent pool (lives for the whole kernel) ----------
    with tc.tile_pool(name="persist", bufs=1) as persist, \
         tc.tile_pool(name="qkvp", bufs=3 * CT) as qkvp, \
         tc.tile_pool(name="otp", bufs=CT) as otp, \
         tc.tile_pool(name="psw", bufs=6, space="PSUM") as psw, \
         tc.tile_pool(name="pss", bufs=1, space="PSUM") as pss:

        cons = persist.tile([128, 6 * 128], bf16, name="cons", tag="cons")
        nc.sync.dma_start(cons[:], consts.ap())
        ident = cons[:, 0:128]          # identity
        m_bdl = cons[:, 128:256]        # block-diag(32) strict lower, +1
        m_bdu = cons[:, 256:384]        # block-diag(32) strict upper, +1
        m_bdln = cons[:, 384:512]       # block-diag(32) strict lower, -1
        m_fneg = cons[:, 512:640]       # strict upper outside blocks, -1
        m_triuI = cons[:, 640:768]      # i<=j, +1
        ones_col = cons[:, 767:768]     # last col of (i<=j) mask == all ones

        biases = persist.tile([128, 3], dt.float32, name="biases", tag="biases")
        nc.vector.memset(biases[:, 0:1], 1e-6)
        nc.vector.memset(biases[:, 1:2], EPS)
        nc.vector.memset(biases[:, 2:3], 1e-6 * DH)

        cwt = []
        for ct in range(CT):
            t_ = persist.tile([128, 3 * CONV_K], fp32, name=f"cw{ct}",
                              tag=f"cw{ct}")
            nc.sync.dma_start(t_[:], cw_t[ct])
            cwt.append(t_)

        qh, kh, vh = [], [], []
        for lst, nm in ((qh, "q"), (kh, "k"), (vh, "v")):
            for ct in range(CT):
                lst.append(qkvp.tile([128, T], bf16, name=f"{nm}hat{ct}",
                                     tag="qkv"))
        oT = [otp.tile([128, T], bf16, name=f"oT{ct}", tag="oT")
              for ct in range(CT)]

        # ================= phase A: projections + conv + silu + l2norm ====
        with tc.tile_pool(name="xp", bufs=KT) as xp, \
             tc.tile_pool(name="wp", bufs=3 * KT) as wp, \
             tc.tile_pool(name="rawp", bufs=2) as rawp, \
             tc.tile_pool(name="sqp", bufs=4) as sqp, \
             tc.tile_pool(name="stp", bufs=1) as stp, \
             tc.tile_pool(name="bcp", bufs=2) as bcp:

            xt = []
            for kt in range(KT):
                t_ = xp.tile([128, T], bf16, name=f"xt{kt}", tag="xt")
                nc.sync.dma_start(t_[:], xT_t[kt])
                xt.append(t_)
            ws = {}
            for nm in ("q", "k", "v"):
                ws[nm] = []
                for kt in range(KT):
                    t_ = wp.tile([128, CG], bf16, name=f"w{nm}{kt}", tag="w")
                    nc.sync.dma_start(t_[:], w_t[nm][kt])
                    ws[nm].append(t_)

            for ti, (nm, dest) in enumerate((("q", qh), ("k", kh), ("v", vh))):
                sq_tiles = []
                for ct in range(CT):
                    rawt = rawp.tile([128, NT], bf16, name=f"raw{nm}{ct}",
                                     tag="raw")
                    nc.vector.memset(rawt[:, 0:PAD], 0.0)
                    dst = dest[ct]
                    for nb in range(NB):
                        pt = psw.tile([128, TOKB], fp32, name=f"pp{nm}{ct}{nb}",
                                      tag="w")
                        for kt in range(KT):
                            nc.tensor.matmul(
                                pt[:], ws[nm][kt][:, ct * 128:(ct + 1) * 128],
                                xt[kt][:, nb * TOKB:(nb + 1) * TOKB],
                                start=(kt == 0), stop=(kt == KT - 1))
                        nc.scalar.copy(
                            rawt[:, PAD + nb * TOKB:PAD + (nb + 1) * TOKB],
                            pt[:])
                    # causal depthwise conv along t
                    w0 = cwt[ct][:, ti * CONV_K:ti * CONV_K + 1]
                    nc.vector.tensor_scalar_mul(dst[:], rawt[:, 1:1 + T], w0)
                    for i in range(1, CONV_K):
                        wi = cwt[ct][:, ti * CONV_K + i:ti * CONV_K + i + 1]
                        nc.vector.scalar_tensor_tensor(
                            dst[:], rawt[:, 1 + i:1 + i + T], wi, dst[:],
                            ALU.mult, ALU.add)
                    if SILU_NATIVE:
                        nc.scalar.activation(dst[:], dst[:], AF.Silu)
                    else:
                        sg = rawp.tile([128, T], bf16, name=f"sg{nm}{ct}",
                                       tag="raw")
                        nc.scalar.activation(sg[:], dst[:], AF.Sigmoid)
                        nc.vector.tensor_mul(dst[:], dst[:], sg[:])
                    if ti < 2:
                        sqt = sqp.tile([128, T], bf16, name=f"sq{nm}{ct}",
                                       tag="sq")
                        nc.scalar.activation(sqt[:], dst[:], AF.Square)
                        sq_tiles.append(sqt)
                if ti < 2:
                    # per-head l2norm: sumsq rows via ones-matmul, broadcast
                    # to 128 partitions, rsq = scale/sqrt(ss + 1e-6), apply.
                    for head in range(2):
                        bcf = bcp.tile([128, T], fp32, name=f"bcf{nm}{head}",
                                       tag="bcf")
                        for nb in range(NB):
                            prow = psw.tile([1, TOKB], fp32,
                                            name=f"pr{nm}{head}{nb}", tag="w")
                            for cth in range(2):
                                nc.tensor.matmul(
                                    prow[:], ones_col,
                                    sq_tiles[head * 2 + cth][
                                        :, nb * TOKB:(nb + 1) * TOKB],
                                    start=(cth == 0), stop=(cth == 1))
                            rowb = stp.tile([1, TOKB], fp32,
                                            name=f"rb{nm}{head}{nb}",
                                            tag="rowb", bufs=3)
                            nc.scalar.copy(rowb[:], prow[:])
                            nc.gpsimd.partition_broadcast(
                                bcf[:, nb * TOKB:(nb + 1) * TOKB], rowb[:])
                        if ti == 0:
                            # fold Dh^-0.5: 1/(16 sqrt(ss+eps)) =
                            # 1/sqrt(256 ss + 256 eps)
                            nc.scalar.activation(bcf[:], bcf[:], AF.Sqrt,
                                                 bias=biases[:, 2:3],
                                                 scale=float(DH))
                        else:
                            nc.scalar.activation(bcf[:], bcf[:], AF.Sqrt,
                                                 bias=biases[:, 0:1])
                        nc.vector.reciprocal(bcf[:], bcf[:])
                        bcb = bcp.tile([128, T], bf16, name=f"bcb{nm}{head}",
                                       tag="bcb")
                        nc.scalar.copy(bcb[:], bcf[:])
                        for cth in range(2):
                            ct = head * 2 + cth
                            nc.vector.tensor_mul(dest[ct][:], dest[ct][:],
                                                 bcb[:])

        # ================= phase B: delta-rule recurrence =================
        with tc.tile_pool(name="recp", bufs=4) as recp, \
             tc.tile_pool(name="recs", bufs=1) as recs:
            for head in range(2):
                ct0 = head * 2
                s_ps = pss.tile([128, 512], fp32, name=f"sps{head}", tag="sps")
                s_sb = recs.tile([128, 512], bf16, name=f"ssb{head}", tag="ssb",
                                 bufs=2)
                nc.vector.memset(s_sb[:], 0.0)
                for ch in range(NCHUNK):
                    t0 = ch * C
                    QT = [qh[ct0][:, t0:t0 + C], qh[ct0 + 1][:, t0:t0 + C]]
                    KTt = [kh[ct0][:, t0:t0 + C], kh[ct0 + 1][:, t0:t0 + C]]
                    VT = [vh[ct0][:, t0:t0 + C], vh[ct0 + 1][:, t0:t0 + C]]

                    # K, V in [C, Dh] layout via PE transpose (bf16 psum)
                    kcd = recp.tile([128, 256], bf16, name=f"kcd{head}{ch}",
                                    tag="kcd")
                    vcd = recp.tile([128, 256], bf16, name=f"vcd{head}{ch}",
                                    tag="vcd")
                    for i in range(2):
                        ptk = psw.tile([128, 128], bf16, name=f"ptk{head}{ch}{i}",
                                       tag="w")
                        nc.tensor.transpose(ptk[:], KTt[i], ident)
                        nc.scalar.copy(kcd[:, i * 128:(i + 1) * 128], ptk[:])
                        ptv = psw.tile([128, 128], bf16, name=f"ptv{head}{ch}{i}",
                                       tag="w")
                        nc.tensor.transpose(ptv[:], VT[i], ident)
                        nc.scalar.copy(vcd[:, i * 128:(i + 1) * 128], ptv[:])

                    # KK^T; A, B, -A masks
                    pkk = psw.tile([128, 128], fp32, name=f"pkk{head}{ch}",
                                   tag="w")
                    for i in range(2):
                        nc.tensor.matmul(pkk[:], KTt[i], KTt[i], start=(i == 0),
                                         stop=(i == 1))
                    Nl = recp.tile([128, 128], bf16, name=f"Nl{head}{ch}",
                                   tag="Nl")
                    Nln = recp.tile([128, 128], bf16, name=f"Nln{head}{ch}",
                                    tag="Nln")
                    Nu = recp.tile([128, 128], bf16, name=f"Nu{head}{ch}",
                                   tag="Nu")
                    FnT = recp.tile([128, 128], bf16, name=f"FnT{head}{ch}",
                                    tag="FnT")
                    nc.vector.tensor_mul(Nl[:], pkk[:], m_bdl)
                    nc.vector.tensor_mul(Nu[:], pkk[:], m_bdu)
                    nc.vector.tensor_mul(Nln[:], pkk[:], m_bdln)
                    nc.vector.tensor_mul(FnT[:], pkk[:], m_fneg)

                    # R = D^T = (I+Nu)^{-1}, block-diag(32): 4 exact levels
                    pR = psw.tile([128, 128], fp32, name=f"pR{head}{ch}",
                                  tag="w")
                    nc.tensor.matmul(pR[:], ident, ident, start=True,
                                     stop=False)
                    nc.tensor.matmul(pR[:], Nln[:], ident, start=False,
                                     stop=True)
                    Rm = recp.tile([128, 128], bf16, name=f"Rm{head}{ch}0",
                                   tag="Rm")
                    nc.scalar.copy(Rm[:], pR[:])
                    Pm, Qm = Nl, Nu
                    for lvl in range(3):
                        pp = psw.tile([128, 128], fp32,
                                      name=f"pp{head}{ch}{lvl}", tag="w")
                        nc.tensor.matmul(pp[:], Qm[:], Pm[:], start=True,
                                         stop=True)
                        Pn = recp.tile([128, 128], bf16,
                                       name=f"Pn{head}{ch}{lvl}", tag="Pn")
                        nc.scalar.copy(Pn[:], pp[:])
                        if lvl < 2:
                            pq = psw.tile([128, 128], fp32,
                                          name=f"pq{head}{ch}{lvl}", tag="w")
                            nc.tensor.matmul(pq[:], Pm[:], Qm[:], start=True,
                                             stop=True)
                            Qn = recp.tile([128, 128], bf16,
                                           name=f"Qn{head}{ch}{lvl}", tag="Qn")
                            nc.scalar.copy(Qn[:], pq[:])
                        else:
                            Qn = None
                        nc.tensor.matmul(pR[:], Pn[:], Rm[:], start=False,
                                         stop=True,
                                         skip_group_check=True)
                        Rn = recp.tile([128, 128], bf16,
                                       name=f"Rm{head}{ch}{lvl + 1}", tag="Rm")
                        nc.scalar.copy(Rn[:], pR[:])
                        Rm, Pm, Qm = Rn, Pn, Qn

                    # RHS' = V - K S    (psum = K@S, then V - psum on DVE)
                    pks = psw.tile([128, 256], fp32, name=f"pks{head}{ch}",
                                   tag="w")
                    for i in range(2):
                        nc.tensor.matmul(pks[:], KTt[i],
                                         s_sb[:, i * 256:(i + 1) * 256],
                                         start=(i == 0), stop=(i == 1))
                    rhs_sb = recp.tile([128, 256], bf16, name=f"rhs{head}{ch}",
                                       tag="rhs")
                    nc.vector.tensor_sub(rhs_sb[:], vcd[:], pks[:])

                    # U' via block forward substitution (4 blocks of 32)
                    u_sb = recp.tile([128, 256], bf16, name=f"u{head}{ch}",
                                     tag="u")
                    y_sb = recp.tile([128, 256], bf16, name=f"y{head}{ch}",
                                     tag="y")
                    nc.vector.memset(u_sb[:], 0.0)
                    px = psw.tile([128, 256], fp32, name=f"px{head}{ch}",
                                  tag="w")
                    py = psw.tile([128, 256], fp32, name=f"py{head}{ch}",
                                  tag="w")
                    nc.tensor.matmul(px[0:32, :], Rm[0:32, 0:32],
                                     rhs_sb[0:32, :], start=True, stop=True,
                                     tile_position=(0, 0))
                    nc.vector.tensor_copy(u_sb[0:32, :], px[0:32, :])
                    for i in range(1, 4):
                        p0 = 32 * i
                        nc.tensor.matmul(py[p0:p0 + 32, :],
                                         FnT[:, p0:p0 + 32], u_sb[:],
                                         start=True, stop=True,
                                         tile_position=(0, p0))
                        nc.vector.tensor_add(y_sb[p0:p0 + 32, :],
                                             rhs_sb[p0:p0 + 32, :],
                                             py[p0:p0 + 32, :])
                        nc.tensor.matmul(px[p0:p0 + 32, :],
                                         Rm[p0:p0 + 32, p0:p0 + 32],
                                         y_sb[p0:p0 + 32, :],
                                         start=True, stop=True,
                                         tile_position=(p0, p0))
                        nc.vector.tensor_copy(u_sb[p0:p0 + 32, :],
                                              px[p0:p0 + 32, :])

                    # attn P = triu_incl(K Q^T)
                    pkq = psw.tile([128, 128], fp32, name=f"pkq{head}{ch}",
                                   tag="w")
                    for i in range(2):
                        nc.tensor.matmul(pkq[:], KTt[i], QT[i], start=(i == 0),
                                         stop=(i == 1))
                    Pat = recp.tile([128, 128], bf16, name=f"Pat{head}{ch}",
                                    tag="Pat")
                    nc.vector.tensor_mul(Pat[:], pkq[:], m_triuI)

                    # O = Q S + P^T U'
                    po = psw.tile([128, 256], fp32, name=f"po{head}{ch}",
                                  tag="w")
                    for i in range(2):
                        nc.tensor.matmul(po[:], QT[i],
                                         s_sb[:, i * 256:(i + 1) * 256],
                                         start=(i == 0), stop=False)
                    nc.tensor.matmul(po[:], Pat[:], u_sb[:], start=False,
                                     stop=True)

                    # S += K^T U'   (accumulate in persistent psum)
                    for i in range(2):
                        nc.tensor.matmul(s_ps[:, i * 256:(i + 1) * 256],
                                         kcd[:, i * 128:(i + 1) * 128], u_sb[:],
                                         start=(ch == 0 and i == 0), stop=True,
                                         skip_group_check=True)
                    s_nb = recs.tile([128, 512], bf16, name=f"ssb{head}{ch}",
                                     tag="ssb", bufs=2)
                    nc.vector.tensor_copy(s_nb[:], s_ps[:])
                    s_sb = s_nb

                    # RMSNorm rows of O, then transpose out to oT
                    osq = recp.tile([128, 256], bf16, name=f"osq{head}{ch}",
                                    tag="osq")
                    ossq = recp.tile([128, 1], fp32, name=f"ossq{head}{ch}",
                                     tag="ossq")
                    nc.scalar.activation(osq[:], po[:], AF.Square,
                                         accum_out=ossq[:])
                    orsq = recp.tile([128, 1], fp32, name=f"orsq{head}{ch}",
                                     tag="orsq")
                    nc.scalar.activation(orsq[:], ossq[:], AF.Sqrt,
                                         bias=biases[:, 1:2], scale=1.0 / DH)
                    nc.vector.reciprocal(orsq[:], orsq[:])
                    onrm = recp.tile([128, 256], bf16, name=f"onrm{head}{ch}",
                                     tag="onrm")
                    nc.vector.tensor_scalar_mul(onrm[:], po[:], orsq[:])
                    for i in range(2):
                        pto = psw.tile([128, 128], bf16,
                                       name=f"pto{head}{ch}{i}", tag="w")
                        nc.tensor.transpose(pto[:], onrm[:, i * 128:(i + 1) * 128],
                                            ident)
                        nc.scalar.copy(oT[ct0 + i][:, t0:t0 + C], pto[:])

        # ================= phase C: output projection =====================
        if DEBUG_SKIP_WO:
            return
        with tc.tile_pool(name="wop", bufs=CT) as wop, \
             tc.tile_pool(name="ofp", bufs=3) as ofp:
            wo_s = []
            for ct in range(CT):
                t_ = wop.tile([128, D], bf16, name=f"wo{ct}", tag="wo")
                nc.sync.dma_start(t_[:], wo_t[ct])
                wo_s.append(t_)
            for tt in range(T // 128):
                for half in range(2):
                    pf = psw.tile([128, 512], fp32, name=f"pf{tt}{half}",
                                  tag="w")
                    for ct in range(CT):
                        nc.tensor.matmul(
                            pf[:], oT[ct][:, tt * 128:(tt + 1) * 128],
                            wo_s[ct][:, half * 512:(half + 1) * 512],
                            start=(ct == 0), stop=(ct == CT - 1))
                    of = ofp.tile([128, 512], fp32, name=f"of{tt}{half}",
                                  tag="of")
                    nc.scalar.copy(of[:], pf[:])
                    nc.sync.dma_start(
                        out_t[tt][:, half * 512:(half + 1) * 512], of[:])


LP_NP = np.float16  # host-side 16-bit dtype matching the device dtype


def _make_consts():
    ii = np.arange(128)
    blk = ii[:, None] // 32 == ii[None, :] // 32
    ident = np.eye(128, dtype=np.float32)
    bdl = ((ii[:, None] > ii[None, :]) & blk).astype(np.float32)
    bdu = ((ii[:, None] < ii[None, :]) & blk).astype(np.float32)
    fneg = -((ii[:, None] < ii[None, :]) & ~blk).astype(np.float32)
    triuI = (ii[:, None] <= ii[None, :]).astype(np.float32)
    return np.concatenate([ident, bdl, bdu, -bdl, fneg, triuI],
                          axis=1).astype(LP_NP)


def _get_compiled():
    key = ("nc", SILU_NATIVE)
    if key not in _CACHE:
        _CACHE[key] = _build_bass()
    return _CACHE[key]


def kernel(hidden_states, Wq, Wk, Wv, conv_wq, conv_wk, conv_wv, onorm_w, Wo):
    from concourse.bass_utils import run_bass_kernel_spmd

    hidden_states = np.asarray(hidden_states, np.float32)
    Wq = np.asarray(Wq, np.float32)
    Wk = np.asarray(Wk, np.float32)
    Wv = np.asarray(Wv, np.float32)
    Wo = np.asarray(Wo, np.float32)
    conv_wq = np.asarray(conv_wq, np.float32)
    conv_wk = np.asarray(conv_wk, np.float32)
    conv_wv = np.asarray(conv_wv, np.float32)
    onorm_w = np.asarray(onorm_w, np.float32)

    bf = LP_NP
    consts = _make_consts()
    Wo_eff = (Wo * np.tile(onorm_w, H)[:, None]).astype(bf)  # fold RMS weight

    in_maps = []
    for core in range(NCORES):
        b, g = divmod(core, 2)
        cols = slice(CG * g, CG * (g + 1))
        in_maps.append({
            "xT": np.ascontiguousarray(hidden_states[b].T).astype(bf),
            "wq": np.ascontiguousarray(Wq[:, cols]).astype(bf),
            "wk": np.ascontiguousarray(Wk[:, cols]).astype(bf),
            "wv": np.ascontiguousarray(Wv[:, cols]).astype(bf),
            "wo": np.ascontiguousarray(Wo_eff[cols, :]),
            "cw": np.ascontiguousarray(np.concatenate(
                [conv_wq[cols], conv_wk[cols], conv_wv[cols]], axis=1)),
            "consts": consts,
        })

    nc = _get_compiled()
    res = run_bass_kernel_spmd(nc, in_maps, core_ids=list(range(NCORES)),
                               **_CACHE.get("run_kwargs", {}))
    _CACHE["last_results"] = res
    out = np.zeros((B, T, D), np.float32)
    for core in range(NCORES):
        out[core // 2] += res.results[core]["out"]
    return out



# revision 34
# speedup vs baseline: 1.8241x; 1.8241x over previous
"""DeltaNet forward kernel for 8 Trainium2 NeuronCores (v3).

Problem (hardcoded): hidden_states [B=4, T=2048, D=1024], H=4 heads, Dh=256,
causal depthwise conv K=4 + silu on q/k/v projections, q/k l2-normalized per
head (q scaled Dh^-0.5), delta-rule recurrence over T, per-head RMSNorm,
merge heads, out = o @ Wo.

Sharding: core c -> batch c//2, head group c%2 (512 projection columns).
Each core computes a partial product against its 512 rows of Wo; the host
sums the two partials per batch.

Design vs baseline:
- q l2norm folded into the output RMSNorm bias:
  out = o_raw / sqrt(mean(o_raw^2) + 256*EPS*|q_raw|^2) (exact up to 2.56e-9).
- Chunked delta rule (C=128) with the chunk inverse computed densely:
  RT = (I+B)^-1 (B = strict upper of K K^T) via 4-level Neumann doubling
  using the transposed-pair trick (track P=B^2^k and P^T together so every
  matmul has its stationary operand pre-transposed). Exponents <= 31;
  validated 1e-4 (f64) / ~3e-3 (fp16) against the exact recurrence.
- Per chunk precompute [Z|W] = R [V|K]; the S-dependent critical path is
  only: pks = W S -> u = Z - pks -> S += K^T u -> copy S (4 hops).
- Both heads interleaved per chunk; head-paired elementwise ops in the
  R chain; phase A runs in 2 halves with half 1 spliced between chunks
  0..7; the output projection streams per 128-token chunk.
- fp16 everywhere (fp8 tested: quantization error does not average down
  for random-sign dot products -> ~4% output error, over budget).
- Activation-table discipline: Copy/Square are in every act table; Silu
  and Sqrt never share one. All Silus batched so tables load ~4x total.
"""

import numpy as np

B, T, D = 4, 2048, 1024
H = 4
DH = D // H          # 256
CONV_K = 4
EPS = 1e-5
NCORES = 8
CG = 512             # columns per core (2 heads)
C = 128              # recurrence chunk length
NCHUNK = T // C      # 16
PAD = 4              # leading zero pad for causal conv
TOKB = 512           # projection token block (psum width)
HALF = 1024          # conv/norm granularity
NLVL = 4             # doubling levels (exponents <= 2^(NLVL+1)-1 = 31)
KT = 8               # contraction tiles for projections
QBS = float(EPS * DH)   # 2.56e-3: q-sumsq scale folded into RMS bias

_CACHE = {}
DBG = False

# tap0 engine per (ti, ct) flat index 0..11: 1 = Act (Copy*scale), 0 = DVE
CONV_ENG = [1] * 12


def _build_bass():
    import concourse.bass as bass  # noqa: F401
    import concourse.bacc as bacc
    import concourse.mybir as mybir
    import concourse.tile as tile

    dt = mybir.dt
    nc = bacc.Bacc("TRN2", target_bir_lowering=False, debug=False)

    xT = nc.dram_tensor("xT", [D, T], dt.float16, kind="ExternalInput")
    wq = nc.dram_tensor("wq", [D, CG], dt.float16, kind="ExternalInput")
    wk = nc.dram_tensor("wk", [D, CG], dt.float16, kind="ExternalInput")
    wv = nc.dram_tensor("wv", [D, CG], dt.float16, kind="ExternalInput")
    wo = nc.dram_tensor("wo", [CG, D], dt.float16, kind="ExternalInput")
    cw = nc.dram_tensor("cw", [4, 128, 3 * CONV_K], dt.float32,
                        kind="ExternalInput")
    consts = nc.dram_tensor("consts", [128, 1152], dt.float16,
                            kind="ExternalInput")
    out = nc.dram_tensor("out", [T, D], dt.float16, kind="ExternalOutput")
    dbg = nc.dram_tensor("dbg", [128, 5120], dt.float32,
                         kind="ExternalOutput") if DBG else None

    with tile.TileContext(nc) as tc:
        _body(nc, tc, mybir, xT, wq, wk, wv, wo, cw, consts, out, dbg)

    nc.compile()
    return nc


def _body(nc, tc, mybir, xT, wq, wk, wv, wo, cw, consts, out, dbg=None):
    dt = mybir.dt
    AF = mybir.ActivationFunctionType
    ALU = mybir.AluOpType
    fp32 = dt.float32
    f16 = dt.float16

    xT_t = xT.ap().rearrange("(n p) t -> n p t", p=128)      # [8,128,T]
    w_t = {"q": wq.ap().rearrange("(n p) c -> n p c", p=128),
           "k": wk.ap().rearrange("(n p) c -> n p c", p=128),
           "v": wv.ap().rearrange("(n p) c -> n p c", p=128)}
    wo_t = wo.ap().rearrange("(n p) c -> n p c", p=128)      # [4,128,D]
    cw_t = cw.ap()                                           # [4,128,12]
    out_t = out.ap().rearrange("(n p) d -> n p d", p=128)    # [16,128,D]

    bw = [None]   # bwork pool, created after xwp release

    with tc.tile_pool(name="persist", bufs=1) as persist, \
         tc.tile_pool(name="qkvp", bufs=1) as qkvp, \
         tc.tile_pool(name="rawp", bufs=1) as rawp, \
         tc.tile_pool(name="sqp", bufs=1) as sqp, \
         tc.tile_pool(name="normp", bufs=2) as normp, \
         tc.tile_pool(name="ofp", bufs=3) as ofp, \
         tc.tile_pool(name="bigps", bufs=2, space="PSUM") as bigps, \
         tc.tile_pool(name="rps", bufs=2, space="PSUM") as rps, \
         tc.tile_pool(name="kps", bufs=2, space="PSUM") as kps, \
         tc.tile_pool(name="tps", bufs=2, space="PSUM") as tps:

        # ---------------- loads ----------------
        xwp = tc.alloc_tile_pool(name="xwp", bufs=1)
        cons = persist.tile([128, 1152], f16, name="cons", tag="cons")
        nc.sync.dma_start(cons[:], consts.ap())
        ident = cons[:, 0:128]        # I
        m_su2 = cons[:, 128:384]      # [+1 a<b] twice (head-pair masks)
        m_sl2 = cons[:, 384:640]      # [+1 a>b] twice
        m_R02 = cons[:, 640:896]      # [I - strict-upper] twice
        m_tri2 = cons[:, 896:1152]    # [+1 a<=b] twice
        ones_col = cons[:, 1023:1024]  # last col of triuI mask == all ones

        bias6 = persist.tile([128, 1], fp32, name="bias6", tag="bias6")
        nc.vector.memset(bias6[:], 1e-6)

        cwt = []
        for ct in range(4):
            t_ = persist.tile([128, 3 * CONV_K], fp32, name=f"cw{ct}",
                              tag=f"cw{ct}")
            nc.sync.dma_start(t_[:], cw_t[ct])
            cwt.append(t_)

        xt = []
        for kt in range(KT):
            t_ = xwp.tile([128, T], f16, name=f"xt{kt}", tag=f"xt{kt}")
            nc.sync.dma_start(t_[:], xT_t[kt])
            xt.append(t_)
        ws = {}
        for nm in ("q", "k", "v"):
            ws[nm] = []
            for kt in range(KT):
                t_ = xwp.tile([128, CG], f16, name=f"w{nm}{kt}",
                              tag=f"w{nm}{kt}")
                nc.sync.dma_start(t_[:], w_t[nm][kt])
                ws[nm].append(t_)
        wlist = [ws["q"], ws["k"], ws["v"]]
        wo_s = []
        for ct in range(4):
            t_ = persist.tile([128, D], f16, name=f"wos{ct}", tag=f"wos{ct}")
            nc.sync.dma_start(t_[:], wo_t[ct])
            wo_s.append(t_)

        # ---------------- persistent working tensors ----------------
        # qkh[ct]: [q | k] over time; vh[ct]: v; oTp[h]: output^T pair layout
        qkh = [qkvp.tile([128, 2 * T], f16, name=f"qkh{ct}", tag=f"qkh{ct}")
               for ct in range(4)]
        vh = [qkvp.tile([128, T], f16, name=f"vh{ct}", tag=f"vh{ct}")
              for ct in range(4)]
        oTp = [qkvp.tile([128, 2 * T], f16, name=f"oTp{h}", tag=f"oTp{h}")
               for h in range(2)]
        raw = [rawp.tile([128, HALF + PAD], f16, name=f"raw{i}", tag=f"raw{i}")
               for i in range(12)]
        for i in range(12):
            nc.gpsimd.memset(raw[i][:, 0:PAD], 0.0)

        s_sb = [None, None]

        # diag(conv weight) tiles for the v-projection conv-as-matmul
        dgv = []
        for ct in range(4):
            row = []
            for i in range(CONV_K):
                d_ = persist.tile([128, 128], f16, name=f"dgv{ct}{i}",
                                  tag=f"dgv{ct}{i}")
                nc.vector.tensor_scalar_mul(
                    d_[:], ident, cwt[ct][:, 2 * CONV_K + i:2 * CONV_K + i + 1])
                row.append(d_)
            dgv.append(row)

        # ============ phase A emission (per half) ============
        def emit_proj_block(half, nb):
            """Projection matmuls + psum->raw copies for one 512-token block."""
            gb = 2 * half + nb
            for ti in range(3):
                for ct in range(4):
                    idx = ti * 4 + ct
                    pp = bigps.tile([128, TOKB], fp32, name=f"pp{gb}{idx}",
                                    tag="big")
                    for kt in range(KT):
                        nc.tensor.matmul(
                            pp[:], wlist[ti][kt][:, ct * 128:(ct + 1) * 128],
                            xt[kt][:, gb * TOKB:(gb + 1) * TOKB],
                            start=(kt == 0), stop=(kt == KT - 1))
                    dst = raw[idx][:, PAD + nb * TOKB:PAD + (nb + 1) * TOKB]
                    if idx % 2 == 0:
                        nc.scalar.copy(dst, pp[:])
                    else:
                        nc.vector.tensor_copy(dst, pp[:])

        def _conv_dst(half, ti, ct):
            t0 = half * HALF
            if ti == 0:
                return qkh[ct][:, t0:t0 + HALF]
            if ti == 1:
                return qkh[ct][:, T + t0:T + t0 + HALF]
            return vh[ct][:, t0:t0 + HALF]

        def emit_conv_taps(half, ti, ct):
            """Causal conv (4 taps) for one (proj, ct) over one half.
            Silu is emitted separately to batch activation-table usage.
            v tiles (ti==2) run the conv on the PE as accumulating
            diag-weight matmuls, with Silu consuming the psum directly."""
            idx = ti * 4 + ct
            dst = _conv_dst(half, ti, ct)
            if ti == 2:
                for nb in range(2):
                    cv = bigps.tile([128, TOKB], fp32, name=f"cv{half}{ct}{nb}",
                                    tag="big")
                    for i in range(CONV_K):
                        nc.tensor.matmul(
                            cv[:], dgv[ct][i],
                            raw[idx][:, 1 + i + nb * TOKB:
                                     1 + i + nb * TOKB + TOKB],
                            start=(i == 0), stop=(i == CONV_K - 1))
                    nc.scalar.activation(
                        dst[:, nb * TOKB:(nb + 1) * TOKB], cv[:], AF.Silu)
                if half == 0:
                    nc.gpsimd.tensor_copy(raw[idx][:, 0:PAD],
                                          raw[idx][:, HALF:HALF + PAD])
                return
            w0 = cwt[ct][:, ti * CONV_K:ti * CONV_K + 1]
            nc.scalar.activation(dst, raw[idx][:, 1:1 + HALF], AF.Copy,
                                 scale=w0)
            tta = sqp.tile([128, HALF], f16, name=f"cta{half}{idx}", tag="cta",
                           bufs=3)
            ttb = sqp.tile([128, HALF], f16, name=f"ctb{half}{idx}", tag="ctb",
                           bufs=3)
            w1 = cwt[ct][:, ti * CONV_K + 1:ti * CONV_K + 2]
            w2 = cwt[ct][:, ti * CONV_K + 2:ti * CONV_K + 3]
            w3 = cwt[ct][:, ti * CONV_K + 3:ti * CONV_K + 4]
            nc.vector.tensor_scalar_mul(tta[:], raw[idx][:, 2:2 + HALF], w1)
            nc.vector.tensor_scalar_mul(ttb[:], raw[idx][:, 3:3 + HALF], w2)
            nc.vector.tensor_add(tta[:], tta[:], ttb[:])
            nc.vector.tensor_scalar_mul(ttb[:], raw[idx][:, 4:4 + HALF], w3)
            nc.vector.tensor_add(dst, dst, tta[:])
            nc.vector.tensor_add(dst, dst, ttb[:])
            # boundary carry for next half (tokens 1020..1023 -> cols 0..3)
            if half == 0:
                nc.gpsimd.tensor_copy(raw[idx][:, 0:PAD],
                                      raw[idx][:, HALF:HALF + PAD])

        def emit_silu(half, ti, ct):
            if ti == 2:
                return
            dst = _conv_dst(half, ti, ct)
            nc.scalar.activation(dst, dst, AF.Silu)

        sq_q = {}   # (half, ct) -> [128, HALF] q^2 tiles for the RMS bias
        def emit_norms(half):
            """k l2norm (+ sq_q tiles) for one half."""
            t0 = half * HALF
            for head in range(2):
                sqk = []
                for i in range(2):
                    ct = 2 * head + i
                    t_ = sqp.tile([128, HALF], f16, name=f"sqk{half}{ct}",
                                  tag="cta", bufs=3)
                    ks = qkh[ct][:, T + t0:T + t0 + HALF]
                    nc.gpsimd.tensor_mul(t_[:], ks, ks)
                    sqk.append(t_)
                bcf = normp.tile([128, HALF], fp32, name=f"bcf{half}{head}",
                                 tag="bcf", bufs=1)
                for nb in range(2):
                    prow = bigps.tile([1, TOKB], fp32,
                                      name=f"pr{half}{head}{nb}", tag="big")
                    for i in range(2):
                        nc.tensor.matmul(prow[:], ones_col,
                                         sqk[i][:, nb * TOKB:(nb + 1) * TOKB],
                                         start=(i == 0), stop=(i == 1))
                    rowb = normp.tile([1, TOKB], fp32,
                                      name=f"rb{half}{head}{nb}", tag="rowb",
                                      bufs=3)
                    nc.scalar.copy(rowb[:], prow[:])
                    nc.gpsimd.partition_broadcast(
                        bcf[:, nb * TOKB:(nb + 1) * TOKB], rowb[:])
                nc.scalar.activation(bcf[:], bcf[:], AF.Sqrt,
                                     bias=bias6[:, 0:1])
                nc.vector.reciprocal(bcf[:], bcf[:])
                bcb = normp.tile([128, HALF], f16, name=f"bcb{half}{head}",
                                 tag="bcb")
                nc.gpsimd.tensor_copy(bcb[:], bcf[:])
                for i in range(2):
                    ct = 2 * head + i
                    ks = qkh[ct][:, T + t0:T + t0 + HALF]
                    nc.gpsimd.tensor_mul(ks, ks, bcb[:])
            for ct in range(4):
                t_ = sqp.tile([128, HALF], f16, name=f"sqq{half}{ct}",
                              tag=f"sqq{ct}", bufs=2)
                qs = qkh[ct][:, t0:t0 + HALF]
                nc.gpsimd.tensor_mul(t_[:], qs, qs)
                sq_q[(half, ct)] = t_

        # ============ phase B emission: software-pipelined stages ============
        # PSUM rings (bank-granular, 8 banks):
        #   bigps x2: pp/prow (phase A), zw, pf
        #   rps  x2: rp [P2 pair | PT2 pair], dac [acc pair]
        #   kps  x2: qps, pkkq, pks, ksu0, ksu1, po
        #   tps  x2: kvt (f16 x4), wot (WT + oT, f16 x4)
        # Iteration k emits chunk k's precompute (R doubling etc.) with chunk
        # k-1's chain/output stages spliced between the R levels, so every
        # engine has ready work queued during the R ping-pong latencies.
        ST = {}

        def st_pre(ch):
            t0 = ch * C
            half = ch // 8
            st = ST[ch] = {}
            kvt = tps.tile([128, 1024], f16, name=f"kvt{ch}", tag="tps")
            qps_t = kps.tile([128, 2], fp32, name=f"qps{ch}", tag="kps")
            pkkq = kps.tile([128, 512], fp32, name=f"pkkq{ch}", tag="kps")
            rp = rps.tile([128, 512], fp32, name=f"rp{ch}", tag="rps")
            Bp = bw[0].tile([128, 256], f16, name=f"Bp{ch}", tag="Bp")
            Ap = bw[0].tile([128, 256], f16, name=f"Ap{ch}", tag="Ap")
            R0p = bw[0].tile([128, 256], f16, name=f"R0p{ch}", tag="Rp", bufs=4)
            rhs_kv = [None, None]
            Pat = [None, None]
            for h in range(2):
                ct0 = 2 * h
                for srcv in range(2):  # 0: v, 1: k
                    for i in range(2):
                        if srcv == 0:
                            ap = vh[ct0 + i][:, t0:t0 + C]
                        else:
                            ap = qkh[ct0 + i][:, T + t0:T + t0 + C]
                        o0 = 512 * h + 256 * srcv + 128 * i
                        nc.tensor.transpose(kvt[:, o0:o0 + 128], ap, ident)
                rkv = bw[0].tile([128, 512], f16, name=f"rkv{ch}{h}", tag="rkv",
                                 bufs=4)
                nc.scalar.copy(rkv[:], kvt[:, 512 * h:512 * (h + 1)])
                rhs_kv[h] = rkv
                pk = pkkq[:, 256 * h:256 * (h + 1)]
                for i in range(2):
                    qk2 = qkh[ct0 + i].rearrange(
                        "p (n t) -> p n t", n=2)[:, :, t0:t0 + C]
                    nc.tensor.matmul(pk, qkh[ct0 + i][:, T + t0:T + t0 + C],
                                     qk2, start=(i == 0), stop=(i == 1))
                qps = qps_t[:, h:h + 1]
                for i in range(2):
                    nc.tensor.matmul(qps, sq_q[(half, ct0 + i)][
                        :, t0 - half * HALF:t0 - half * HALF + C],
                        ones_col, start=(h == 0 and i == 0), stop=(i == 1),
                        skip_group_check=True)
            qbp = bw[0].tile([128, 2], fp32, name=f"qb{ch}", tag="qb", bufs=4)
            nc.scalar.activation(qbp[:], qps_t[:], AF.Copy, scale=QBS)
            # head-paired mask ops ([h0|h1] strided reads of pkkq)
            pkk2 = pkkq.rearrange("p (h c) -> p h c", h=2)[:, :, 128:256]
            pkq2 = pkkq.rearrange("p (h c) -> p h c", h=2)[:, :, 0:128]
            B2 = Bp.rearrange("p (h c) -> p h c", h=2)
            A2_ = Ap.rearrange("p (h c) -> p h c", h=2)
            M2 = m_su2.rearrange("p (h c) -> p h c", h=2)
            nc.vector.tensor_mul(B2, pkk2, M2)
            nc.vector.tensor_mul(A2_, pkk2,
                                 m_sl2.rearrange("p (h c) -> p h c", h=2))
            for h in range(2):
                hs = slice(128 * h, 128 * (h + 1))
                nc.vector.tensor_sub(R0p[:, hs], ident, Bp[:, hs])
            Patp = bw[0].tile([128, 256], f16, name=f"Pat{ch}", tag="Pat",
                              bufs=4)
            nc.vector.tensor_mul(Patp.rearrange("p (h c) -> p h c", h=2),
                                 pkq2, m_tri2.rearrange("p (h c) -> p h c", h=2))
            Pat = [Patp[:, 0:128], Patp[:, 128:256]]
            st.update(rhs_kv=rhs_kv, Pat=Pat, qb=[qbp[:, 0:1], qbp[:, 1:2]],
                      rp=rp, RT=R0p, Pm=Bp, PTm=Ap)

        def st_rlvl(ch, lvl):
            st = ST[ch]
            rp, RT, Pm, PTm = st["rp"], st["RT"], st["Pm"], st["PTm"]
            for h in range(2):
                hs = slice(128 * h, 128 * (h + 1))
                if lvl < NLVL - 1:
                    nc.tensor.matmul(rp[:, hs], PTm[:, hs], Pm[:, hs],
                                     start=True, stop=True,
                                     skip_group_check=True)
                nc.tensor.matmul(rp[:, 256 + 128 * h:256 + 128 * (h + 1)],
                                 Pm[:, hs], PTm[:, hs], start=True,
                                 stop=True, skip_group_check=True)
            PTn = bw[0].tile([128, 256], f16, name=f"ptn{ch}{lvl}", tag="PT",
                             bufs=4)
            nc.vector.tensor_copy(PTn[:], rp[:, 256:512])
            if lvl < NLVL - 1:
                Pn = bw[0].tile([128, 256], f16, name=f"pn{ch}{lvl}", tag="P",
                                bufs=4)
                nc.scalar.copy(Pn[:], rp[:, 0:256])
            else:
                Pn = None
            for h in range(2):
                hs = slice(128 * h, 128 * (h + 1))
                nc.tensor.matmul(rp[:, hs], PTn[:, hs], RT[:, hs],
                                 start=True, stop=True, skip_group_check=True)
            RTn = bw[0].tile([128, 256], f16, name=f"rt{ch}{lvl}", tag="Rp",
                             bufs=4)
            nc.vector.tensor_add(RTn[:], RT[:], rp[:, 0:256])
            st.update(RT=RTn, Pm=Pn, PTm=PTn)

        def st_zw(ch):
            st = ST[ch]
            RT, rhs_kv = st["RT"], st["rhs_kv"]
            zwp = bw[0].tile([128, 1024], f16, name=f"zwp{ch}", tag="zw")
            wtp = tps.tile([128, 512], f16, name=f"wtp{ch}", tag="tps")
            for h in range(2):
                zw = bigps.tile([128, 512], fp32, name=f"zw{ch}{h}", tag="big")
                nc.tensor.matmul(zw[:], RT[:, 128 * h:128 * (h + 1)],
                                 rhs_kv[h][:], start=True, stop=True)
                if h == 0:
                    nc.vector.tensor_copy(zwp[:, 0:512], zw[:])
                else:
                    nc.scalar.copy(zwp[:, 512:1024], zw[:])
            for h in range(2):
                for i in range(2):
                    nc.tensor.transpose(
                        wtp[:, 256 * h + 128 * i:256 * h + 128 * (i + 1)],
                        zwp[:, 512 * h + 256 + 128 * i:
                            512 * h + 256 + 128 * (i + 1)],
                        ident)
            wts = bw[0].tile([128, 512], f16, name=f"wts{ch}", tag="wt")
            nc.scalar.copy(wts[:], wtp[:])
            st.update(zwp=zwp, wts=wts)

        def st_chain1(ch):
            st = ST[ch]
            zwp, wts = st["zwp"], st["wts"]
            s_prev = [s_sb[0], s_sb[1]]
            up = bw[0].tile([128, 512], f16, name=f"up{ch}", tag="u", bufs=4)
            zsel = zwp.rearrange("p (n c) -> p n c", n=4)[:, 0::2, :]
            if ch == 0:
                nc.vector.tensor_copy(
                    up.rearrange("p (n c) -> p n c", n=2), zsel)
            else:
                pks_t = kps.tile([128, 512], fp32, name=f"pks{ch}", tag="kps")
                for h in range(2):
                    pks = pks_t[:, 256 * h:256 * (h + 1)]
                    for i in range(2):
                        nc.tensor.matmul(
                            pks,
                            wts[:, 256 * h + 128 * i:256 * h + 128 * (i + 1)],
                            s_prev[h][:, i * 256:(i + 1) * 256],
                            start=(i == 0), stop=(i == 1))
                nc.vector.tensor_sub(
                    up.rearrange("p (n c) -> p n c", n=2), zsel, pks_t[:])
            st.update(up=up, s_prev=s_prev)

        def st_chain2(ch):
            st = ST[ch]
            up, s_prev, rhs_kv = st["up"], st["s_prev"], st["rhs_kv"]
            for h in range(2):
                ksu = kps.tile([128, 512], fp32, name=f"ksu{ch}{h}", tag="kps")
                for i in range(2):
                    # start once per bank: start=True marks the WHOLE 2KB
                    # bank pending-zero; i=1's start=False write overwrites
                    # its still-pending half (init semantics).
                    nc.tensor.matmul(
                        ksu[:, i * 256:(i + 1) * 256],
                        rhs_kv[h][:, 256 + 128 * i:256 + 128 * (i + 1)],
                        up[:, 256 * h:256 * (h + 1)],
                        start=(i == 0), stop=True, skip_group_check=True)
                s_n = bw[0].tile([128, 512], f16, name=f"ssb{ch}{h}",
                                 tag="ssb", bufs=4)
                if ch == 0:
                    if h == 0:
                        nc.vector.tensor_copy(s_n[:], ksu[:])
                    else:
                        nc.scalar.copy(s_n[:], ksu[:])
                else:
                    nc.vector.tensor_add(s_n[:], s_prev[h][:], ksu[:])
                s_sb[h] = s_n

        def st_o1(ch):
            st = ST[ch]
            t0 = ch * C
            up, s_prev, Pat, qb = st["up"], st["s_prev"], st["Pat"], st["qb"]
            po_t = rps.tile([128, 512], fp32, name=f"po{ch}", tag="rps")
            onrm = bw[0].tile([128, 512], f16, name=f"onrm{ch}", tag="onrm")
            for h in range(2):
                ct0 = 2 * h
                po = po_t[:, 256 * h:256 * (h + 1)]
                if ch == 0:
                    nc.tensor.matmul(po, Pat[h],
                                     up[:, 256 * h:256 * (h + 1)],
                                     start=(h == 0), stop=True,
                                     skip_group_check=True)
                else:
                    for i in range(2):
                        nc.tensor.matmul(po, qkh[ct0 + i][:, t0:t0 + C],
                                         s_prev[h][:, i * 256:(i + 1) * 256],
                                         start=(h == 0 and i == 0), stop=False,
                                         skip_group_check=True)
                    nc.tensor.matmul(po, Pat[h],
                                     up[:, 256 * h:256 * (h + 1)],
                                     start=False, stop=True,
                                     skip_group_check=True)
                osq = bw[0].tile([128, 256], f16, name=f"osq{ch}{h}",
                                 tag="osq")
                ossq = bw[0].tile([128, 1], fp32, name=f"ossq{ch}{h}",
                                  tag="ossq", bufs=4)
                nc.scalar.activation(osq[:], po, AF.Square, accum_out=ossq[:])
                orsq = bw[0].tile([128, 1], fp32, name=f"orsq{ch}{h}",
                                  tag="orsq", bufs=4)
                nc.scalar.activation(orsq[:], ossq[:], AF.Sqrt,
                                     bias=qb[h], scale=1.0 / DH)
                nc.vector.reciprocal(orsq[:], orsq[:])
                nc.vector.tensor_scalar_mul(onrm[:, 256 * h:256 * (h + 1)],
                                            po, orsq[:])
            st.update(onrm=onrm)

        def st_o2c(ch):
            st = ST[ch]
            t0 = ch * C
            onrm = st["onrm"]
            otp = tps.tile([128, 512], f16, name=f"otp{ch}", tag="tps")
            for h in range(2):
                for i in range(2):
                    nc.tensor.transpose(
                        otp[:, 256 * h + 128 * i:256 * h + 128 * (i + 1)],
                        onrm[:, 256 * h + i * 128:256 * h + (i + 1) * 128],
                        ident)
                dstp = oTp[h].rearrange("p (n t) -> p n t",
                                        n=2)[:, :, t0:t0 + C]
                srcp = otp[:, 256 * h:256 * (h + 1)].rearrange(
                    "p (n t) -> p n t", n=2)
                nc.scalar.activation(dstp, srcp, AF.Copy)
            for hf in range(2):
                pf = bigps.tile([128, 512], fp32, name=f"pf{ch}{hf}",
                                tag="big")
                for ct in range(4):
                    h, i = divmod(ct, 2)
                    nc.tensor.matmul(
                        pf[:], oTp[h][:, i * T + t0:i * T + t0 + C],
                        wo_s[ct][:, hf * 512:(hf + 1) * 512],
                        start=(ct == 0), stop=(ct == 3))
                of = ofp.tile([128, 512], f16, name=f"of{ch}{hf}", tag="of")
                if hf == 0:
                    nc.vector.tensor_copy(of[:], pf[:])
                else:
                    nc.scalar.copy(of[:], pf[:])
                nc.sync.dma_start(out_t[ch][:, hf * 512:(hf + 1) * 512], of[:])
            del ST[ch]

        # ============ top-level emission order ============
        emit_proj_block(0, 0)
        emit_proj_block(0, 1)
        for ti in range(3):
            for ct in range(4):
                emit_conv_taps(0, ti, ct)
                emit_silu(0, ti, ct)
        emit_norms(0)
        emit_proj_block(1, 0)
        emit_proj_block(1, 1)
        # projection inputs are dead now; reuse their SBUF for phase B work
        xwp.release()
        bw[0] = tc.alloc_tile_pool(name="bwork", bufs=3)
        for h in range(2):
            t_ = bw[0].tile([128, 512], f16, name=f"ssb{h}_init", tag="ssb",
                            bufs=4)
            nc.vector.memset(t_[:], 0.0)
            s_sb[h] = t_

        def _silus_norms():
            for ti in range(3):
                for ct in range(4):
                    emit_silu(1, ti, ct)
            emit_norms(1)

        a1 = [
            lambda: [emit_conv_taps(1, 0, ct) for ct in range(4)],
            lambda: [emit_conv_taps(1, 1, ct) for ct in range(4)],
            lambda: [emit_conv_taps(1, 2, ct) for ct in range(4)],
            _silus_norms,
        ]

        for it in range(NCHUNK // 2 + 1):
            c0, c1 = 2 * it, 2 * it + 1
            p0, p1 = c0 - 2, c1 - 2
            pre = c0 < NCHUNK
            if pre:
                st_pre(c0)
                st_pre(c1)
                st_rlvl(c0, 0)
                st_rlvl(c1, 0)
            if p0 >= 0:
                st_chain1(p0)
            if pre:
                st_rlvl(c0, 1)
                st_rlvl(c1, 1)
            if p0 >= 0:
                st_chain2(p0)
                st_chain1(p1)
            if pre:
                st_rlvl(c0, 2)
                st_rlvl(c1, 2)
            if p0 >= 0:
                st_chain2(p1)
                st_o1(p0)
            if pre:
                st_rlvl(c0, 3)
                st_rlvl(c1, 3)
            if p0 >= 0:
                st_o1(p1)
                st_o2c(p0)
                st_o2c(p1)
            if pre:
                st_zw(c0)
                st_zw(c1)
            if it < len(a1):
                a1[it]()
        bw[0].release()


LP_NP = np.float16


def _make_consts():
    ii = np.arange(128)
    ident = np.eye(128, dtype=np.float32)
    m_su = (ii[:, None] < ii[None, :]).astype(np.float32)
    m_sl = (ii[:, None] > ii[None, :]).astype(np.float32)
    m_R0 = ident - m_su
    m_triuI = (ii[:, None] <= ii[None, :]).astype(np.float32)
    return np.concatenate([ident, m_su, m_su, m_sl, m_sl, m_R0, m_R0,
                           m_triuI, m_triuI], axis=1).astype(LP_NP)


def _get_compiled():
    if "nc" not in _CACHE:
        _CACHE["nc"] = _build_bass()
    return _CACHE["nc"]


def kernel(hidden_states, Wq, Wk, Wv, conv_wq, conv_wk, conv_wv, onorm_w, Wo):
    from concourse.bass_utils import run_bass_kernel_spmd

    hidden_states = np.asarray(hidden_states, np.float32)
    Wq = np.asarray(Wq, np.float32)
    Wk = np.asarray(Wk, np.float32)
    Wv = np.asarray(Wv, np.float32)
    Wo = np.asarray(Wo, np.float32)
    conv_wq = np.asarray(conv_wq, np.float32)
    conv_wk = np.asarray(conv_wk, np.float32)
    conv_wv = np.asarray(conv_wv, np.float32)
    onorm_w = np.asarray(onorm_w, np.float32)

    consts = _make_consts()
    Wo_eff = (Wo * np.tile(onorm_w, H)[:, None]).astype(LP_NP)

    in_maps = []
    for core in range(NCORES):
        b, g = divmod(core, 2)
        cols = slice(CG * g, CG * (g + 1))
        cwf = np.concatenate([conv_wq[cols], conv_wk[cols], conv_wv[cols]],
                             axis=1)
        in_maps.append({
            "xT": np.ascontiguousarray(hidden_states[b].T).astype(LP_NP),
            "wq": np.ascontiguousarray(Wq[:, cols]).astype(LP_NP),
            "wk": np.ascontiguousarray(Wk[:, cols]).astype(LP_NP),
            "wv": np.ascontiguousarray(Wv[:, cols]).astype(LP_NP),
            "wo": np.ascontiguousarray(Wo_eff[cols, :]),
            "cw": np.ascontiguousarray(cwf.reshape(4, 128, 3 * CONV_K)),
            "consts": consts,
        })

    nc = _get_compiled()
    res = run_bass_kernel_spmd(nc, in_maps, core_ids=list(range(NCORES)),
                               **_CACHE.get("run_kwargs", {}))
    _CACHE["last_results"] = res
    out = np.zeros((B, T, D), np.float32)
    for core in range(NCORES):
        out[core // 2] += res.results[core]["out"].astype(np.float32)
    return out


# revision 35
# speedup vs baseline: 1.8770x; 1.0290x over previous
"""DeltaNet forward kernel for 8 Trainium2 NeuronCores (v3).

Problem (hardcoded): hidden_states [B=4, T=2048, D=1024], H=4 heads, Dh=256,
causal depthwise conv K=4 + silu on q/k/v projections, q/k l2-normalized per
head (q scaled Dh^-0.5), delta-rule recurrence over T, per-head RMSNorm,
merge heads, out = o @ Wo.

Sharding: core c -> batch c//2, head group c%2 (512 projection columns).
Each core computes a partial product against its 512 rows of Wo; the host
sums the two partials per batch.

Design vs baseline:
- q l2norm folded into the output RMSNorm bias:
  out = o_raw / sqrt(mean(o_raw^2) + 256*EPS*|q_raw|^2) (exact up to 2.56e-9).
- Chunked delta rule (C=128) with the chunk inverse computed densely:
  RT = (I+B)^-1 (B = strict upper of K K^T) via 4-level Neumann doubling
  using the transposed-pair trick (track P=B^2^k and P^T together so every
  matmul has its stationary operand pre-transposed). Exponents <= 31;
  validated 1e-4 (f64) / ~3e-3 (fp16) against the exact recurrence.
- Per chunk precompute [Z|W] = R [V|K]; the S-dependent critical path is
  only: pks = W S -> u = Z - pks -> S += K^T u -> copy S (4 hops).
- Both heads interleaved per chunk; head-paired elementwise ops in the
  R chain; phase A runs in 2 halves with half 1 spliced between chunks
  0..7; the output projection streams per 128-token chunk.
- fp16 everywhere (fp8 tested: quantization error does not average down
  for random-sign dot products -> ~4% output error, over budget).
- Activation-table discipline: Copy/Square are in every act table; Silu
  and Sqrt never share one. All Silus batched so tables load ~4x total.
"""

import numpy as np

B, T, D = 4, 2048, 1024
H = 4
DH = D // H          # 256
CONV_K = 4
EPS = 1e-5
NCORES = 8
CG = 512             # columns per core (2 heads)
C = 128              # recurrence chunk length
NCHUNK = T // C      # 16
PAD = 4              # leading zero pad for causal conv
TOKB = 512           # projection token block (psum width)
HALF = 1024          # conv/norm granularity
NLVL = 4             # doubling levels (exponents <= 2^(NLVL+1)-1 = 31)
KT = 8               # contraction tiles for projections
QBS = float(EPS * DH)   # 2.56e-3: q-sumsq scale folded into RMS bias

_CACHE = {}
DBG = False

# tap0 engine per (ti, ct) flat index 0..11: 1 = Act (Copy*scale), 0 = DVE
CONV_ENG = [1] * 12


def _build_bass():
    import concourse.bass as bass  # noqa: F401
    import concourse.bacc as bacc
    import concourse.mybir as mybir
    import concourse.tile as tile

    dt = mybir.dt
    nc = bacc.Bacc("TRN2", target_bir_lowering=False, debug=False)

    xT = nc.dram_tensor("xT", [D, T], dt.float16, kind="ExternalInput")
    wq = nc.dram_tensor("wq", [D, CG], dt.float16, kind="ExternalInput")
    wk = nc.dram_tensor("wk", [D, CG], dt.float16, kind="ExternalInput")
    wv = nc.dram_tensor("wv", [D, CG], dt.float16, kind="ExternalInput")
    wo = nc.dram_tensor("wo", [CG, D], dt.float16, kind="ExternalInput")
    cw = nc.dram_tensor("cw", [4, 128, 3 * CONV_K], dt.float32,
                        kind="ExternalInput")
    consts = nc.dram_tensor("consts", [128, 1152], dt.float16,
                            kind="ExternalInput")
    out = nc.dram_tensor("out", [T, D], dt.float16, kind="ExternalOutput")
    dbg = nc.dram_tensor("dbg", [128, 5120], dt.float32,
                         kind="ExternalOutput") if DBG else None

    with tile.TileContext(nc) as tc:
        _body(nc, tc, mybir, xT, wq, wk, wv, wo, cw, consts, out, dbg)

    nc.compile()
    return nc


def _body(nc, tc, mybir, xT, wq, wk, wv, wo, cw, consts, out, dbg=None):
    dt = mybir.dt
    AF = mybir.ActivationFunctionType
    ALU = mybir.AluOpType
    fp32 = dt.float32
    f16 = dt.float16

    xT_t = xT.ap().rearrange("(n p) t -> n p t", p=128)      # [8,128,T]
    w_t = {"q": wq.ap().rearrange("(n p) c -> n p c", p=128),
           "k": wk.ap().rearrange("(n p) c -> n p c", p=128),
           "v": wv.ap().rearrange("(n p) c -> n p c", p=128)}
    wo_t = wo.ap().rearrange("(n p) c -> n p c", p=128)      # [4,128,D]
    cw_t = cw.ap()                                           # [4,128,12]
    out_t = out.ap().rearrange("(n p) d -> n p d", p=128)    # [16,128,D]

    bw = [None]   # bwork pool, created after xwp release

    with tc.tile_pool(name="persist", bufs=1) as persist, \
         tc.tile_pool(name="qkvp", bufs=1) as qkvp, \
         tc.tile_pool(name="rawp", bufs=1) as rawp, \
         tc.tile_pool(name="sqp", bufs=1) as sqp, \
         tc.tile_pool(name="normp", bufs=2) as normp, \
         tc.tile_pool(name="ofp", bufs=3) as ofp, \
         tc.tile_pool(name="bigps", bufs=2, space="PSUM") as bigps, \
         tc.tile_pool(name="rps", bufs=2, space="PSUM") as rps, \
         tc.tile_pool(name="kps", bufs=2, space="PSUM") as kps, \
         tc.tile_pool(name="tps", bufs=2, space="PSUM") as tps:

        # ---------------- loads ----------------
        xwp = tc.alloc_tile_pool(name="xwp", bufs=1)
        cons = persist.tile([128, 1152], f16, name="cons", tag="cons")
        nc.sync.dma_start(cons[:], consts.ap())
        ident = cons[:, 0:128]        # I
        m_su2 = cons[:, 128:384]      # [+1 a<b] twice (head-pair masks)
        m_sl2 = cons[:, 384:640]      # [+1 a>b] twice
        m_R02 = cons[:, 640:896]      # [I - strict-upper] twice
        m_tri2 = cons[:, 896:1152]    # [+1 a<=b] twice
        ones_col = cons[:, 1023:1024]  # last col of triuI mask == all ones

        bias6 = persist.tile([128, 1], fp32, name="bias6", tag="bias6")
        nc.vector.memset(bias6[:], 1e-6)

        cwt = []
        for ct in range(4):
            t_ = persist.tile([128, 3 * CONV_K], fp32, name=f"cw{ct}",
                              tag=f"cw{ct}")
            nc.sync.dma_start(t_[:], cw_t[ct])
            cwt.append(t_)

        xt = []
        for kt in range(KT):
            t_ = xwp.tile([128, T], f16, name=f"xt{kt}", tag=f"xt{kt}")
            nc.sync.dma_start(t_[:], xT_t[kt])
            xt.append(t_)
        ws = {}
        for nm in ("q", "k", "v"):
            ws[nm] = []
            for kt in range(KT):
                t_ = xwp.tile([128, CG], f16, name=f"w{nm}{kt}",
                              tag=f"w{nm}{kt}")
                nc.sync.dma_start(t_[:], w_t[nm][kt])
                ws[nm].append(t_)
        wlist = [ws["q"], ws["k"], ws["v"]]
        wo_s = []
        for ct in range(4):
            t_ = persist.tile([128, D], f16, name=f"wos{ct}", tag=f"wos{ct}")
            nc.sync.dma_start(t_[:], wo_t[ct])
            wo_s.append(t_)

        # ---------------- persistent working tensors ----------------
        # qkh[ct]: [q | k] over time; vh[ct]: v; oTp[h]: output^T pair layout
        qkh = [qkvp.tile([128, 2 * T], f16, name=f"qkh{ct}", tag=f"qkh{ct}")
               for ct in range(4)]
        vh = [qkvp.tile([128, T], f16, name=f"vh{ct}", tag=f"vh{ct}")
              for ct in range(4)]
        oTp = [qkvp.tile([128, 2 * T], f16, name=f"oTp{h}", tag=f"oTp{h}")
               for h in range(2)]
        raw = [rawp.tile([128, HALF + PAD], f16, name=f"raw{i}", tag=f"raw{i}")
               for i in range(12)]
        for i in range(12):
            nc.gpsimd.memset(raw[i][:, 0:PAD], 0.0)

        s_sb = [None, None]

        # diag(conv weight) tiles for the v-projection conv-as-matmul
        dgv = []
        for ct in range(4):
            row = []
            for i in range(CONV_K):
                d_ = persist.tile([128, 128], f16, name=f"dgv{ct}{i}",
                                  tag=f"dgv{ct}{i}")
                nc.vector.tensor_scalar_mul(
                    d_[:], ident, cwt[ct][:, 2 * CONV_K + i:2 * CONV_K + i + 1])
                row.append(d_)
            dgv.append(row)

        # ============ phase A emission (per half) ============
        def emit_proj_block(half, nb):
            """Projection matmuls + psum->raw copies for one 512-token block."""
            gb = 2 * half + nb
            for ti in range(3):
                for ct in range(4):
                    idx = ti * 4 + ct
                    pp = bigps.tile([128, TOKB], fp32, name=f"pp{gb}{idx}",
                                    tag="big")
                    for kt in range(KT):
                        nc.tensor.matmul(
                            pp[:], wlist[ti][kt][:, ct * 128:(ct + 1) * 128],
                            xt[kt][:, gb * TOKB:(gb + 1) * TOKB],
                            start=(kt == 0), stop=(kt == KT - 1))
                    dst = raw[idx][:, PAD + nb * TOKB:PAD + (nb + 1) * TOKB]
                    if idx % 2 == 0:
                        nc.scalar.copy(dst, pp[:])
                    else:
                        nc.vector.tensor_copy(dst, pp[:])

        def _conv_dst(half, ti, ct):
            t0 = half * HALF
            if ti == 0:
                return qkh[ct][:, t0:t0 + HALF]
            if ti == 1:
                return qkh[ct][:, T + t0:T + t0 + HALF]
            return vh[ct][:, t0:t0 + HALF]

        def emit_conv_taps(half, ti, ct):
            """Causal conv (4 taps) for one (proj, ct) over one half.
            Silu is emitted separately to batch activation-table usage.
            v tiles (ti==2) run the conv on the PE as accumulating
            diag-weight matmuls, with Silu consuming the psum directly."""
            idx = ti * 4 + ct
            dst = _conv_dst(half, ti, ct)
            if ti == 2:
                for nb in range(2):
                    cv = bigps.tile([128, TOKB], fp32, name=f"cv{half}{ct}{nb}",
                                    tag="big")
                    for i in range(CONV_K):
                        nc.tensor.matmul(
                            cv[:], dgv[ct][i],
                            raw[idx][:, 1 + i + nb * TOKB:
                                     1 + i + nb * TOKB + TOKB],
                            start=(i == 0), stop=(i == CONV_K - 1))
                    nc.scalar.activation(
                        dst[:, nb * TOKB:(nb + 1) * TOKB], cv[:], AF.Silu)
                if half == 0:
                    nc.gpsimd.tensor_copy(raw[idx][:, 0:PAD],
                                          raw[idx][:, HALF:HALF + PAD])
                return
            w0 = cwt[ct][:, ti * CONV_K:ti * CONV_K + 1]
            nc.scalar.activation(dst, raw[idx][:, 1:1 + HALF], AF.Copy,
                                 scale=w0)
            tta = sqp.tile([128, HALF], f16, name=f"cta{half}{idx}", tag="cta",
                           bufs=3)
            ttb = sqp.tile([128, HALF], f16, name=f"ctb{half}{idx}", tag="ctb",
                           bufs=3)
            w1 = cwt[ct][:, ti * CONV_K + 1:ti * CONV_K + 2]
            w2 = cwt[ct][:, ti * CONV_K + 2:ti * CONV_K + 3]
            w3 = cwt[ct][:, ti * CONV_K + 3:ti * CONV_K + 4]
            nc.vector.tensor_scalar_mul(tta[:], raw[idx][:, 2:2 + HALF], w1)
            nc.vector.tensor_scalar_mul(ttb[:], raw[idx][:, 3:3 + HALF], w2)
            nc.vector.tensor_add(tta[:], tta[:], ttb[:])
            nc.vector.tensor_scalar_mul(ttb[:], raw[idx][:, 4:4 + HALF], w3)
            nc.vector.tensor_add(dst, dst, tta[:])
            nc.vector.tensor_add(dst, dst, ttb[:])
            # boundary carry for next half (tokens 1020..1023 -> cols 0..3)
            if half == 0:
                nc.gpsimd.tensor_copy(raw[idx][:, 0:PAD],
                                      raw[idx][:, HALF:HALF + PAD])

        def emit_silu(half, ti, ct):
            if ti == 2:
                return
            dst = _conv_dst(half, ti, ct)
            nc.scalar.activation(dst, dst, AF.Silu)

        sq_q = {}   # (half, ct) -> [128, HALF] q^2 tiles for the RMS bias
        def emit_norms(half):
            """k l2norm (+ sq_q tiles) for one half."""
            t0 = half * HALF
            etn = nc.gpsimd if half == 0 else nc.vector
            for ct in range(4):
                t_ = sqp.tile([128, HALF], f16, name=f"sqq{half}{ct}",
                              tag=f"sqq{ct}", bufs=2)
                qs = qkh[ct][:, t0:t0 + HALF]
                etn.tensor_mul(t_[:], qs, qs)
                sq_q[(half, ct)] = t_
            for head in range(2):
                sqk = []
                for i in range(2):
                    ct = 2 * head + i
                    t_ = sqp.tile([128, HALF], f16, name=f"sqk{half}{ct}",
                                  tag="cta", bufs=3)
                    ks = qkh[ct][:, T + t0:T + t0 + HALF]
                    etn.tensor_mul(t_[:], ks, ks)
                    sqk.append(t_)
                bcf = normp.tile([128, HALF], fp32, name=f"bcf{half}{head}",
                                 tag="bcf", bufs=1)
                for nb in range(2):
                    prow = bigps.tile([1, TOKB], fp32,
                                      name=f"pr{half}{head}{nb}", tag="big")
                    for i in range(2):
                        nc.tensor.matmul(prow[:], ones_col,
                                         sqk[i][:, nb * TOKB:(nb + 1) * TOKB],
                                         start=(i == 0), stop=(i == 1))
                    rowb = normp.tile([1, TOKB], fp32,
                                      name=f"rb{half}{head}{nb}", tag="rowb",
                                      bufs=3)
                    nc.scalar.copy(rowb[:], prow[:])
                    nc.gpsimd.partition_broadcast(
                        bcf[:, nb * TOKB:(nb + 1) * TOKB], rowb[:])
                nc.scalar.activation(bcf[:], bcf[:], AF.Sqrt,
                                     bias=bias6[:, 0:1])
                nc.vector.reciprocal(bcf[:], bcf[:])
                bcb = normp.tile([128, HALF], f16, name=f"bcb{half}{head}",
                                 tag="bcb")
                etn.tensor_copy(bcb[:], bcf[:])
                for i in range(2):
                    ct = 2 * head + i
                    ks = qkh[ct][:, T + t0:T + t0 + HALF]
                    etn.tensor_mul(ks, ks, bcb[:])

        # ============ phase B emission: software-pipelined stages ============
        # PSUM rings (bank-granular, 8 banks):
        #   bigps x2: pp/prow (phase A), zw, pf
        #   rps  x2: rp [P2 pair | PT2 pair], dac [acc pair]
        #   kps  x2: qps, pkkq, pks, ksu0, ksu1, po
        #   tps  x2: kvt (f16 x4), wot (WT + oT, f16 x4)
        # Iteration k emits chunk k's precompute (R doubling etc.) with chunk
        # k-1's chain/output stages spliced between the R levels, so every
        # engine has ready work queued during the R ping-pong latencies.
        ST = {}

        def st_pre(ch):
            t0 = ch * C
            half = ch // 8
            st = ST[ch] = {}
            kvt = tps.tile([128, 1024], f16, name=f"kvt{ch}", tag="tps")
            qps_t = kps.tile([128, 2], fp32, name=f"qps{ch}", tag="kps")
            pkkq = kps.tile([128, 512], fp32, name=f"pkkq{ch}", tag="kps")
            rp = rps.tile([128, 512], fp32, name=f"rp{ch}", tag="rps")
            Bp = bw[0].tile([128, 256], f16, name=f"Bp{ch}", tag="Bp")
            Ap = bw[0].tile([128, 256], f16, name=f"Ap{ch}", tag="Ap")
            R0p = bw[0].tile([128, 256], f16, name=f"R0p{ch}", tag="Rp", bufs=4)
            rhs_kv = [None, None]
            Pat = [None, None]
            for h in range(2):
                ct0 = 2 * h
                for srcv in range(2):  # 0: v, 1: k
                    for i in range(2):
                        if srcv == 0:
                            ap = vh[ct0 + i][:, t0:t0 + C]
                        else:
                            ap = qkh[ct0 + i][:, T + t0:T + t0 + C]
                        o0 = 512 * h + 256 * srcv + 128 * i
                        nc.tensor.transpose(kvt[:, o0:o0 + 128], ap, ident)
                rkv = bw[0].tile([128, 512], f16, name=f"rkv{ch}{h}", tag="rkv",
                                 bufs=4)
                nc.scalar.copy(rkv[:], kvt[:, 512 * h:512 * (h + 1)])
                rhs_kv[h] = rkv
                pk = pkkq[:, 256 * h:256 * (h + 1)]
                for i in range(2):
                    qk2 = qkh[ct0 + i].rearrange(
                        "p (n t) -> p n t", n=2)[:, :, t0:t0 + C]
                    nc.tensor.matmul(pk, qkh[ct0 + i][:, T + t0:T + t0 + C],
                                     qk2, start=(i == 0), stop=(i == 1))
                qps = qps_t[:, h:h + 1]
                for i in range(2):
                    nc.tensor.matmul(qps, sq_q[(half, ct0 + i)][
                        :, t0 - half * HALF:t0 - half * HALF + C],
                        ones_col, start=(h == 0 and i == 0), stop=(i == 1),
                        skip_group_check=True)
            qbp = bw[0].tile([128, 2], fp32, name=f"qb{ch}", tag="qb", bufs=4)
            nc.scalar.activation(qbp[:], qps_t[:], AF.Copy, scale=QBS)
            # head-paired mask ops ([h0|h1] strided reads of pkkq)
            pkk2 = pkkq.rearrange("p (h c) -> p h c", h=2)[:, :, 128:256]
            pkq2 = pkkq.rearrange("p (h c) -> p h c", h=2)[:, :, 0:128]
            B2 = Bp.rearrange("p (h c) -> p h c", h=2)
            A2_ = Ap.rearrange("p (h c) -> p h c", h=2)
            M2 = m_su2.rearrange("p (h c) -> p h c", h=2)
            nc.vector.tensor_mul(B2, pkk2, M2)
            nc.vector.tensor_mul(A2_, pkk2,
                                 m_sl2.rearrange("p (h c) -> p h c", h=2))
            for h in range(2):
                hs = slice(128 * h, 128 * (h + 1))
                nc.vector.tensor_sub(R0p[:, hs], ident, Bp[:, hs])
            Patp = bw[0].tile([128, 256], f16, name=f"Pat{ch}", tag="Pat",
                              bufs=4)
            nc.vector.tensor_mul(Patp.rearrange("p (h c) -> p h c", h=2),
                                 pkq2, m_tri2.rearrange("p (h c) -> p h c", h=2))
            Pat = [Patp[:, 0:128], Patp[:, 128:256]]
            st.update(rhs_kv=rhs_kv, Pat=Pat, qb=[qbp[:, 0:1], qbp[:, 1:2]],
                      rp=rp, RT=R0p, Pm=Bp, PTm=Ap)

        def st_rlvl(ch, lvl):
            st = ST[ch]
            rp, RT, Pm, PTm = st["rp"], st["RT"], st["Pm"], st["PTm"]
            for h in range(2):
                hs = slice(128 * h, 128 * (h + 1))
                if lvl < NLVL - 1:
                    nc.tensor.matmul(rp[:, hs], PTm[:, hs], Pm[:, hs],
                                     start=True, stop=True,
                                     skip_group_check=True)
                nc.tensor.matmul(rp[:, 256 + 128 * h:256 + 128 * (h + 1)],
                                 Pm[:, hs], PTm[:, hs], start=True,
                                 stop=True, skip_group_check=True)
            PTn = bw[0].tile([128, 256], f16, name=f"ptn{ch}{lvl}", tag="PT",
                             bufs=4)
            nc.vector.tensor_copy(PTn[:], rp[:, 256:512])
            if lvl < NLVL - 1:
                Pn = bw[0].tile([128, 256], f16, name=f"pn{ch}{lvl}", tag="P",
                                bufs=4)
                nc.scalar.copy(Pn[:], rp[:, 0:256])
            else:
                Pn = None
            for h in range(2):
                hs = slice(128 * h, 128 * (h + 1))
                nc.tensor.matmul(rp[:, hs], PTn[:, hs], RT[:, hs],
                                 start=True, stop=True, skip_group_check=True)
            RTn = bw[0].tile([128, 256], f16, name=f"rt{ch}{lvl}", tag="Rp",
                             bufs=4)
            nc.vector.tensor_add(RTn[:], RT[:], rp[:, 0:256])
            st.update(RT=RTn, Pm=Pn, PTm=PTn)

        def st_zw(ch):
            st = ST[ch]
            RT, rhs_kv = st["RT"], st["rhs_kv"]
            zwp = bw[0].tile([128, 1024], f16, name=f"zwp{ch}", tag="zw")
            wtp = tps.tile([128, 512], f16, name=f"wtp{ch}", tag="tps")
            for h in range(2):
                zw = bigps.tile([128, 512], fp32, name=f"zw{ch}{h}", tag="big")
                nc.tensor.matmul(zw[:], RT[:, 128 * h:128 * (h + 1)],
                                 rhs_kv[h][:], start=True, stop=True)
                if h == 0:
                    nc.vector.tensor_copy(zwp[:, 0:512], zw[:])
                else:
                    nc.scalar.copy(zwp[:, 512:1024], zw[:])
            for h in range(2):
                for i in range(2):
                    nc.tensor.transpose(
                        wtp[:, 256 * h + 128 * i:256 * h + 128 * (i + 1)],
                        zwp[:, 512 * h + 256 + 128 * i:
                            512 * h + 256 + 128 * (i + 1)],
                        ident)
            wts = bw[0].tile([128, 512], f16, name=f"wts{ch}", tag="wt")
            nc.scalar.copy(wts[:], wtp[:])
            st.update(zwp=zwp, wts=wts)

        def st_chain1(ch):
            st = ST[ch]
            zwp, wts = st["zwp"], st["wts"]
            s_prev = [s_sb[0], s_sb[1]]
            up = bw[0].tile([128, 512], f16, name=f"up{ch}", tag="u", bufs=4)
            zsel = zwp.rearrange("p (n c) -> p n c", n=4)[:, 0::2, :]
            if ch == 0:
                nc.vector.tensor_copy(
                    up.rearrange("p (n c) -> p n c", n=2), zsel)
            else:
                pks_t = kps.tile([128, 512], fp32, name=f"pks{ch}", tag="kps")
                for h in range(2):
                    pks = pks_t[:, 256 * h:256 * (h + 1)]
                    for i in range(2):
                        nc.tensor.matmul(
                            pks,
                            wts[:, 256 * h + 128 * i:256 * h + 128 * (i + 1)],
                            s_prev[h][:, i * 256:(i + 1) * 256],
                            start=(i == 0), stop=(i == 1))
                nc.vector.tensor_sub(
                    up.rearrange("p (n c) -> p n c", n=2), zsel, pks_t[:])
            st.update(up=up, s_prev=s_prev)

        def st_chain2(ch):
            st = ST[ch]
            up, s_prev, rhs_kv = st["up"], st["s_prev"], st["rhs_kv"]
            for h in range(2):
                ksu = kps.tile([128, 512], fp32, name=f"ksu{ch}{h}", tag="kps")
                for i in range(2):
                    # start once per bank: start=True marks the WHOLE 2KB
                    # bank pending-zero; i=1's start=False write overwrites
                    # its still-pending half (init semantics).
                    nc.tensor.matmul(
                        ksu[:, i * 256:(i + 1) * 256],
                        rhs_kv[h][:, 256 + 128 * i:256 + 128 * (i + 1)],
                        up[:, 256 * h:256 * (h + 1)],
                        start=(i == 0), stop=True, skip_group_check=True)
                s_n = bw[0].tile([128, 512], f16, name=f"ssb{ch}{h}",
                                 tag="ssb", bufs=4)
                if ch == 0:
                    if h == 0:
                        nc.vector.tensor_copy(s_n[:], ksu[:])
                    else:
                        nc.scalar.copy(s_n[:], ksu[:])
                else:
                    nc.vector.tensor_add(s_n[:], s_prev[h][:], ksu[:])
                s_sb[h] = s_n

        def st_o1(ch):
            st = ST[ch]
            t0 = ch * C
            up, s_prev, Pat, qb = st["up"], st["s_prev"], st["Pat"], st["qb"]
            po_t = rps.tile([128, 512], fp32, name=f"po{ch}", tag="rps")
            onrm = bw[0].tile([128, 512], f16, name=f"onrm{ch}", tag="onrm")
            for h in range(2):
                ct0 = 2 * h
                po = po_t[:, 256 * h:256 * (h + 1)]
                if ch == 0:
                    nc.tensor.matmul(po, Pat[h],
                                     up[:, 256 * h:256 * (h + 1)],
                                     start=(h == 0), stop=True,
                                     skip_group_check=True)
                else:
                    for i in range(2):
                        nc.tensor.matmul(po, qkh[ct0 + i][:, t0:t0 + C],
                                         s_prev[h][:, i * 256:(i + 1) * 256],
                                         start=(h == 0 and i == 0), stop=False,
                                         skip_group_check=True)
                    nc.tensor.matmul(po, Pat[h],
                                     up[:, 256 * h:256 * (h + 1)],
                                     start=False, stop=True,
                                     skip_group_check=True)
                osq = bw[0].tile([128, 256], f16, name=f"osq{ch}{h}",
                                 tag="osq")
                ossq = bw[0].tile([128, 1], fp32, name=f"ossq{ch}{h}",
                                  tag="ossq", bufs=4)
                nc.scalar.activation(osq[:], po, AF.Square, accum_out=ossq[:])
                orsq = bw[0].tile([128, 1], fp32, name=f"orsq{ch}{h}",
                                  tag="orsq", bufs=4)
                nc.scalar.activation(orsq[:], ossq[:], AF.Sqrt,
                                     bias=qb[h], scale=1.0 / DH)
                nc.vector.reciprocal(orsq[:], orsq[:])
                nc.vector.tensor_scalar_mul(onrm[:, 256 * h:256 * (h + 1)],
                                            po, orsq[:])
            st.update(onrm=onrm)

        def st_o2c(ch):
            st = ST[ch]
            t0 = ch * C
            onrm = st["onrm"]
            otp = tps.tile([128, 512], f16, name=f"otp{ch}", tag="tps")
            for h in range(2):
                for i in range(2):
                    nc.tensor.transpose(
                        otp[:, 256 * h + 128 * i:256 * h + 128 * (i + 1)],
                        onrm[:, 256 * h + i * 128:256 * h + (i + 1) * 128],
                        ident)
                dstp = oTp[h].rearrange("p (n t) -> p n t",
                                        n=2)[:, :, t0:t0 + C]
                srcp = otp[:, 256 * h:256 * (h + 1)].rearrange(
                    "p (n t) -> p n t", n=2)
                nc.scalar.activation(dstp, srcp, AF.Copy)
            for hf in range(2):
                pf = bigps.tile([128, 512], fp32, name=f"pf{ch}{hf}",
                                tag="big")
                for ct in range(4):
                    h, i = divmod(ct, 2)
                    nc.tensor.matmul(
                        pf[:], oTp[h][:, i * T + t0:i * T + t0 + C],
                        wo_s[ct][:, hf * 512:(hf + 1) * 512],
                        start=(ct == 0), stop=(ct == 3))
                of = ofp.tile([128, 512], f16, name=f"of{ch}{hf}", tag="of")
                if hf == 0:
                    nc.vector.tensor_copy(of[:], pf[:])
                else:
                    nc.scalar.copy(of[:], pf[:])
                nc.sync.dma_start(out_t[ch][:, hf * 512:(hf + 1) * 512], of[:])
            del ST[ch]

        # ============ top-level emission order ============
        emit_proj_block(0, 0)
        emit_proj_block(0, 1)
        for ti in range(3):
            for ct in range(4):
                emit_conv_taps(0, ti, ct)
                emit_silu(0, ti, ct)
        emit_norms(0)
        emit_proj_block(1, 0)
        emit_proj_block(1, 1)
        # projection inputs are dead now; reuse their SBUF for phase B work
        xwp.release()
        bw[0] = tc.alloc_tile_pool(name="bwork", bufs=3)
        for h in range(2):
            t_ = bw[0].tile([128, 512], f16, name=f"ssb{h}_init", tag="ssb",
                            bufs=4)
            nc.vector.memset(t_[:], 0.0)
            s_sb[h] = t_

        def _silus_norms():
            for ti in range(3):
                for ct in range(4):
                    emit_silu(1, ti, ct)
            emit_norms(1)

        a1 = [
            lambda: [emit_conv_taps(1, 0, ct) for ct in range(4)],
            lambda: [emit_conv_taps(1, 1, ct) for ct in range(4)],
            lambda: [emit_conv_taps(1, 2, ct) for ct in range(4)],
            _silus_norms,
        ]

        for it in range(NCHUNK // 2 + 1):
            c0, c1 = 2 * it, 2 * it + 1
            p0, p1 = c0 - 2, c1 - 2
            pre = c0 < NCHUNK
            if pre:
                st_pre(c0)
                st_pre(c1)
                st_rlvl(c0, 0)
                st_rlvl(c1, 0)
            if p0 >= 0:
                st_chain1(p0)
            if pre:
                st_rlvl(c0, 1)
                st_rlvl(c1, 1)
            if p0 >= 0:
                st_chain2(p0)
                st_chain1(p1)
            if pre:
                st_rlvl(c0, 2)
                st_rlvl(c1, 2)
            if p0 >= 0:
                st_chain2(p1)
                st_o1(p0)
            if pre:
                st_rlvl(c0, 3)
                st_rlvl(c1, 3)
            if p0 >= 0:
                st_o1(p1)
                st_o2c(p0)
                st_o2c(p1)
            if pre:
                st_zw(c0)
                st_zw(c1)
            if it < len(a1):
                a1[it]()
        bw[0].release()


LP_NP = np.float16


def _make_consts():
    ii = np.arange(128)
    ident = np.eye(128, dtype=np.float32)
    m_su = (ii[:, None] < ii[None, :]).astype(np.float32)
    m_sl = (ii[:, None] > ii[None, :]).astype(np.float32)
    m_R0 = ident - m_su
    m_triuI = (ii[:, None] <= ii[None, :]).astype(np.float32)
    return np.concatenate([ident, m_su, m_su, m_sl, m_sl, m_R0, m_R0,
                           m_triuI, m_triuI], axis=1).astype(LP_NP)


def _get_compiled():
    if "nc" not in _CACHE:
        _CACHE["nc"] = _build_bass()
    return _CACHE["nc"]


def kernel(hidden_states, Wq, Wk, Wv, conv_wq, conv_wk, conv_wv, onorm_w, Wo):
    from concourse.bass_utils import run_bass_kernel_spmd

    hidden_states = np.asarray(hidden_states, np.float32)
    Wq = np.asarray(Wq, np.float32)
    Wk = np.asarray(Wk, np.float32)
    Wv = np.asarray(Wv, np.float32)
    Wo = np.asarray(Wo, np.float32)
    conv_wq = np.asarray(conv_wq, np.float32)
    conv_wk = np.asarray(conv_wk, np.float32)
    conv_wv = np.asarray(conv_wv, np.float32)
    onorm_w = np.asarray(onorm_w, np.float32)

    consts = _make_consts()
    Wo_eff = (Wo * np.tile(onorm_w, H)[:, None]).astype(LP_NP)

    in_maps = []
    for core in range(NCORES):
        b, g = divmod(core, 2)
        cols = slice(CG * g, CG * (g + 1))
        cwf = np.concatenate([conv_wq[cols], conv_wk[cols], conv_wv[cols]],
                             axis=1)
        in_maps.append({
            "xT": np.ascontiguousarray(hidden_states[b].T).astype(LP_NP),
            "wq": np.ascontiguousarray(Wq[:, cols]).astype(LP_NP),
            "wk": np.ascontiguousarray(Wk[:, cols]).astype(LP_NP),
            "wv": np.ascontiguousarray(Wv[:, cols]).astype(LP_NP),
            "wo": np.ascontiguousarray(Wo_eff[cols, :]),
            "cw": np.ascontiguousarray(cwf.reshape(4, 128, 3 * CONV_K)),
            "consts": consts,
        })

    nc = _get_compiled()
    res = run_bass_kernel_spmd(nc, in_maps, core_ids=list(range(NCORES)),
                               **_CACHE.get("run_kwargs", {}))
    _CACHE["last_results"] = res
    out = np.zeros((B, T, D), np.float32)
    for core in range(NCORES):
        out[core // 2] += res.results[core]["out"].astype(np.float32)
    return out


# revision 36
# speedup vs baseline: 1.8793x; 1.0012x over previous
"""DeltaNet forward kernel for 8 Trainium2 NeuronCores (v3).

Problem (hardcoded): hidden_states [B=4, T=2048, D=1024], H=4 heads, Dh=256,
causal depthwise conv K=4 + silu on q/k/v projections, q/k l2-normalized per
head (q scaled Dh^-0.5), delta-rule recurrence over T, per-head RMSNorm,
merge heads, out = o @ Wo.

Sharding: core c -> batch c//2, head group c%2 (512 projection columns).
Each core computes a partial product against its 512 rows of Wo; the host
sums the two partials per batch.

Design vs baseline:
- q l2norm folded into the output RMSNorm bias:
  out = o_raw / sqrt(mean(o_raw^2) + 256*EPS*|q_raw|^2) (exact up to 2.56e-9).
- Chunked delta rule (C=128) with the chunk inverse computed densely:
  RT = (I+B)^-1 (B = strict upper of K K^T) via 4-level Neumann doubling
  using the transposed-pair trick (track P=B^2^k and P^T together so every
  matmul has its stationary operand pre-transposed). Exponents <= 31;
  validated 1e-4 (f64) / ~3e-3 (fp16) against the exact recurrence.
- Per chunk precompute [Z|W] = R [V|K]; the S-dependent critical path is
  only: pks = W S -> u = Z - pks -> S += K^T u -> copy S (4 hops).
- Both heads interleaved per chunk; head-paired elementwise ops in the
  R chain; phase A runs in 2 halves with half 1 spliced between chunks
  0..7; the output projection streams per 128-token chunk.
- fp16 everywhere (fp8 tested: quantization error does not average down
  for random-sign dot products -> ~4% output error, over budget).
- Activation-table discipline: Copy/Square are in every act table; Silu
  and Sqrt never share one. All Silus batched so tables load ~4x total.
"""

import numpy as np

B, T, D = 4, 2048, 1024
H = 4
DH = D // H          # 256
CONV_K = 4
EPS = 1e-5
NCORES = 8
CG = 512             # columns per core (2 heads)
C = 128              # recurrence chunk length
NCHUNK = T // C      # 16
PAD = 4              # leading zero pad for causal conv
TOKB = 512           # projection token block (psum width)
HALF = 1024          # conv/norm granularity
NLVL = 4             # doubling levels (exponents <= 2^(NLVL+1)-1 = 31)
KT = 8               # contraction tiles for projections
QBS = float(EPS * DH)   # 2.56e-3: q-sumsq scale folded into RMS bias

_CACHE = {}
DBG = False

# tap0 engine per (ti, ct) flat index 0..11: 1 = Act (Copy*scale), 0 = DVE
CONV_ENG = [1] * 12


def _build_bass():
    import concourse.bass as bass  # noqa: F401
    import concourse.bacc as bacc
    import concourse.mybir as mybir
    import concourse.tile as tile

    dt = mybir.dt
    nc = bacc.Bacc("TRN2", target_bir_lowering=False, debug=False)

    xT = nc.dram_tensor("xT", [D, T], dt.float16, kind="ExternalInput")
    wq = nc.dram_tensor("wq", [D, CG], dt.float16, kind="ExternalInput")
    wk = nc.dram_tensor("wk", [D, CG], dt.float16, kind="ExternalInput")
    wv = nc.dram_tensor("wv", [D, CG], dt.float16, kind="ExternalInput")
    wo = nc.dram_tensor("wo", [CG, D], dt.float16, kind="ExternalInput")
    cw = nc.dram_tensor("cw", [4, 128, 3 * CONV_K], dt.float32,
                        kind="ExternalInput")
    consts = nc.dram_tensor("consts", [128, 1152], dt.float16,
                            kind="ExternalInput")
    out = nc.dram_tensor("out", [T, D], dt.float16, kind="ExternalOutput")
    dbg = nc.dram_tensor("dbg", [128, 5120], dt.float32,
                         kind="ExternalOutput") if DBG else None

    with tile.TileContext(nc) as tc:
        _body(nc, tc, mybir, xT, wq, wk, wv, wo, cw, consts, out, dbg)

    nc.compile()
    return nc


def _body(nc, tc, mybir, xT, wq, wk, wv, wo, cw, consts, out, dbg=None):
    dt = mybir.dt
    AF = mybir.ActivationFunctionType
    ALU = mybir.AluOpType
    fp32 = dt.float32
    f16 = dt.float16

    xT_t = xT.ap().rearrange("(n p) t -> n p t", p=128)      # [8,128,T]
    w_t = {"q": wq.ap().rearrange("(n p) c -> n p c", p=128),
           "k": wk.ap().rearrange("(n p) c -> n p c", p=128),
           "v": wv.ap().rearrange("(n p) c -> n p c", p=128)}
    wo_t = wo.ap().rearrange("(n p) c -> n p c", p=128)      # [4,128,D]
    cw_t = cw.ap()                                           # [4,128,12]
    out_t = out.ap().rearrange("(n p) d -> n p d", p=128)    # [16,128,D]

    bw = [None]   # bwork pool, created after xwp release

    with tc.tile_pool(name="persist", bufs=1) as persist, \
         tc.tile_pool(name="qkvp", bufs=1) as qkvp, \
         tc.tile_pool(name="rawp", bufs=1) as rawp, \
         tc.tile_pool(name="sqp", bufs=1) as sqp, \
         tc.tile_pool(name="normp", bufs=2) as normp, \
         tc.tile_pool(name="ofp", bufs=3) as ofp, \
         tc.tile_pool(name="bigps", bufs=2, space="PSUM") as bigps, \
         tc.tile_pool(name="rps", bufs=2, space="PSUM") as rps, \
         tc.tile_pool(name="kps", bufs=2, space="PSUM") as kps, \
         tc.tile_pool(name="tps", bufs=2, space="PSUM") as tps:

        # ---------------- loads ----------------
        xwp = tc.alloc_tile_pool(name="xwp", bufs=1)
        cons = persist.tile([128, 1152], f16, name="cons", tag="cons")
        nc.sync.dma_start(cons[:], consts.ap())
        ident = cons[:, 0:128]        # I
        m_su2 = cons[:, 128:384]      # [+1 a<b] twice (head-pair masks)
        m_sl2 = cons[:, 384:640]      # [+1 a>b] twice
        m_R02 = cons[:, 640:896]      # [I - strict-upper] twice
        m_tri2 = cons[:, 896:1152]    # [+1 a<=b] twice
        ones_col = cons[:, 1023:1024]  # last col of triuI mask == all ones

        bias6 = persist.tile([128, 1], fp32, name="bias6", tag="bias6")
        nc.vector.memset(bias6[:], 1e-6)

        cwt = []
        for ct in range(4):
            t_ = persist.tile([128, 3 * CONV_K], fp32, name=f"cw{ct}",
                              tag=f"cw{ct}")
            nc.sync.dma_start(t_[:], cw_t[ct])
            cwt.append(t_)

        xt = []
        for kt in range(KT):
            t_ = xwp.tile([128, T], f16, name=f"xt{kt}", tag=f"xt{kt}")
            nc.sync.dma_start(t_[:], xT_t[kt])
            xt.append(t_)
        ws = {}
        for nm in ("q", "k", "v"):
            ws[nm] = []
            for kt in range(KT):
                t_ = xwp.tile([128, CG], f16, name=f"w{nm}{kt}",
                              tag=f"w{nm}{kt}")
                nc.sync.dma_start(t_[:], w_t[nm][kt])
                ws[nm].append(t_)
        wlist = [ws["q"], ws["k"], ws["v"]]
        wo_s = []
        for ct in range(4):
            t_ = persist.tile([128, D], f16, name=f"wos{ct}", tag=f"wos{ct}")
            nc.sync.dma_start(t_[:], wo_t[ct])
            wo_s.append(t_)

        # ---------------- persistent working tensors ----------------
        # qkh[ct]: [q | k] over time; vh[ct]: v; oTp[h]: output^T pair layout
        qkh = [qkvp.tile([128, 2 * T], f16, name=f"qkh{ct}", tag=f"qkh{ct}")
               for ct in range(4)]
        vh = [qkvp.tile([128, T], f16, name=f"vh{ct}", tag=f"vh{ct}")
              for ct in range(4)]
        oTp = [qkvp.tile([128, 2 * T], f16, name=f"oTp{h}", tag=f"oTp{h}")
               for h in range(2)]
        raw = [rawp.tile([128, HALF + PAD], f16, name=f"raw{i}", tag=f"raw{i}")
               for i in range(12)]
        for i in range(12):
            nc.gpsimd.memset(raw[i][:, 0:PAD], 0.0)

        s_sb = [None, None]

        # diag(conv weight) tiles for the v-projection conv-as-matmul
        dgv = []
        for ct in range(4):
            row = []
            for i in range(CONV_K):
                d_ = persist.tile([128, 128], f16, name=f"dgv{ct}{i}",
                                  tag=f"dgv{ct}{i}")
                nc.vector.tensor_scalar_mul(
                    d_[:], ident, cwt[ct][:, 2 * CONV_K + i:2 * CONV_K + i + 1])
                row.append(d_)
            dgv.append(row)

        # ============ phase A emission (per half) ============
        def emit_proj_block(half, nb):
            """Projection matmuls + psum->raw copies for one 512-token block."""
            gb = 2 * half + nb
            for ti in range(3):
                for ct in range(4):
                    idx = ti * 4 + ct
                    pp = bigps.tile([128, TOKB], fp32, name=f"pp{gb}{idx}",
                                    tag="big")
                    for kt in range(KT):
                        nc.tensor.matmul(
                            pp[:], wlist[ti][kt][:, ct * 128:(ct + 1) * 128],
                            xt[kt][:, gb * TOKB:(gb + 1) * TOKB],
                            start=(kt == 0), stop=(kt == KT - 1))
                    dst = raw[idx][:, PAD + nb * TOKB:PAD + (nb + 1) * TOKB]
                    if idx % 2 == 0:
                        nc.scalar.copy(dst, pp[:])
                    else:
                        nc.vector.tensor_copy(dst, pp[:])

        def _conv_dst(half, ti, ct):
            t0 = half * HALF
            if ti == 0:
                return qkh[ct][:, t0:t0 + HALF]
            if ti == 1:
                return qkh[ct][:, T + t0:T + t0 + HALF]
            return vh[ct][:, t0:t0 + HALF]

        def emit_conv_taps(half, ti, ct):
            """Causal conv (4 taps) for one (proj, ct) over one half.
            Silu is emitted separately to batch activation-table usage.
            v tiles (ti==2) run the conv on the PE as accumulating
            diag-weight matmuls, with Silu consuming the psum directly."""
            idx = ti * 4 + ct
            dst = _conv_dst(half, ti, ct)
            if ti == 2:
                for nb in range(2):
                    cv = bigps.tile([128, TOKB], fp32, name=f"cv{half}{ct}{nb}",
                                    tag="big")
                    for i in range(CONV_K):
                        nc.tensor.matmul(
                            cv[:], dgv[ct][i],
                            raw[idx][:, 1 + i + nb * TOKB:
                                     1 + i + nb * TOKB + TOKB],
                            start=(i == 0), stop=(i == CONV_K - 1))
                    nc.scalar.activation(
                        dst[:, nb * TOKB:(nb + 1) * TOKB], cv[:], AF.Silu)
                if half == 0:
                    nc.gpsimd.tensor_copy(raw[idx][:, 0:PAD],
                                          raw[idx][:, HALF:HALF + PAD])
                return
            w0 = cwt[ct][:, ti * CONV_K:ti * CONV_K + 1]
            nc.scalar.activation(dst, raw[idx][:, 1:1 + HALF], AF.Copy,
                                 scale=w0)
            tta = sqp.tile([128, HALF], f16, name=f"cta{half}{idx}", tag="cta",
                           bufs=3)
            ttb = sqp.tile([128, HALF], f16, name=f"ctb{half}{idx}", tag="ctb",
                           bufs=3)
            w1 = cwt[ct][:, ti * CONV_K + 1:ti * CONV_K + 2]
            w2 = cwt[ct][:, ti * CONV_K + 2:ti * CONV_K + 3]
            w3 = cwt[ct][:, ti * CONV_K + 3:ti * CONV_K + 4]
            nc.vector.tensor_scalar_mul(tta[:], raw[idx][:, 2:2 + HALF], w1)
            nc.vector.tensor_scalar_mul(ttb[:], raw[idx][:, 3:3 + HALF], w2)
            nc.vector.tensor_add(tta[:], tta[:], ttb[:])
            nc.vector.tensor_scalar_mul(ttb[:], raw[idx][:, 4:4 + HALF], w3)
            nc.vector.tensor_add(dst, dst, tta[:])
            nc.vector.tensor_add(dst, dst, ttb[:])
            # boundary carry for next half (tokens 1020..1023 -> cols 0..3)
            if half == 0:
                nc.gpsimd.tensor_copy(raw[idx][:, 0:PAD],
                                      raw[idx][:, HALF:HALF + PAD])

        def emit_silu(half, ti, ct):
            if ti == 2:
                return
            dst = _conv_dst(half, ti, ct)
            nc.scalar.activation(dst, dst, AF.Silu)

        sq_q = {}   # (half, ct) -> [128, HALF] q^2 tiles for the RMS bias
        def emit_norms(half):
            """k l2norm (+ sq_q tiles) for one half."""
            t0 = half * HALF
            etn = nc.gpsimd if half == 0 else nc.vector
            for ct in range(4):
                t_ = sqp.tile([128, HALF], f16, name=f"sqq{half}{ct}",
                              tag=f"sqq{ct}", bufs=2)
                qs = qkh[ct][:, t0:t0 + HALF]
                etn.tensor_mul(t_[:], qs, qs)
                sq_q[(half, ct)] = t_
            for head in range(2):
                sqk = []
                for i in range(2):
                    ct = 2 * head + i
                    t_ = sqp.tile([128, HALF], f16, name=f"sqk{half}{ct}",
                                  tag="cta", bufs=3)
                    ks = qkh[ct][:, T + t0:T + t0 + HALF]
                    etn.tensor_mul(t_[:], ks, ks)
                    sqk.append(t_)
                bcf = normp.tile([128, HALF], fp32, name=f"bcf{half}{head}",
                                 tag="bcf", bufs=1)
                for nb in range(2):
                    prow = bigps.tile([1, TOKB], fp32,
                                      name=f"pr{half}{head}{nb}", tag="big")
                    for i in range(2):
                        nc.tensor.matmul(prow[:], ones_col,
                                         sqk[i][:, nb * TOKB:(nb + 1) * TOKB],
                                         start=(i == 0), stop=(i == 1))
                    rowb = normp.tile([1, TOKB], fp32,
                                      name=f"rb{half}{head}{nb}", tag="rowb",
                                      bufs=3)
                    nc.scalar.copy(rowb[:], prow[:])
                    nc.gpsimd.partition_broadcast(
                        bcf[:, nb * TOKB:(nb + 1) * TOKB], rowb[:])
                nc.scalar.activation(bcf[:], bcf[:], AF.Sqrt,
                                     bias=bias6[:, 0:1])
                nc.vector.reciprocal(bcf[:], bcf[:])
                bcb = normp.tile([128, HALF], f16, name=f"bcb{half}{head}",
                                 tag="bcb")
                etn.tensor_copy(bcb[:], bcf[:])
                for i in range(2):
                    ct = 2 * head + i
                    ks = qkh[ct][:, T + t0:T + t0 + HALF]
                    etn.tensor_mul(ks, ks, bcb[:])

        # ============ phase B emission: software-pipelined stages ============
        # PSUM rings (bank-granular, 8 banks):
        #   bigps x2: pp/prow (phase A), zw, pf
        #   rps  x2: rp [P2 pair | PT2 pair], dac [acc pair]
        #   kps  x2: qps, pkkq, pks, ksu0, ksu1, po
        #   tps  x2: kvt (f16 x4), wot (WT + oT, f16 x4)
        # Iteration k emits chunk k's precompute (R doubling etc.) with chunk
        # k-1's chain/output stages spliced between the R levels, so every
        # engine has ready work queued during the R ping-pong latencies.
        ST = {}

        def st_pre(ch):
            t0 = ch * C
            half = ch // 8
            st = ST[ch] = {}
            kvt = tps.tile([128, 1024], f16, name=f"kvt{ch}", tag="tps")
            qps_t = kps.tile([128, 2], fp32, name=f"qps{ch}", tag="kps")
            pkkq = kps.tile([128, 512], fp32, name=f"pkkq{ch}", tag="kps")
            rp = rps.tile([128, 512], fp32, name=f"rp{ch}", tag="rps")
            Bp = bw[0].tile([128, 256], f16, name=f"Bp{ch}", tag="Bp")
            Ap = bw[0].tile([128, 256], f16, name=f"Ap{ch}", tag="Ap")
            R0p = bw[0].tile([128, 256], f16, name=f"R0p{ch}", tag="Rp", bufs=4)
            rhs_kv = [None, None]
            Pat = [None, None]
            for h in range(2):
                ct0 = 2 * h
                for srcv in range(2):  # 0: v, 1: k
                    for i in range(2):
                        if srcv == 0:
                            ap = vh[ct0 + i][:, t0:t0 + C]
                        else:
                            ap = qkh[ct0 + i][:, T + t0:T + t0 + C]
                        o0 = 512 * h + 256 * srcv + 128 * i
                        nc.tensor.transpose(kvt[:, o0:o0 + 128], ap, ident)
                rkv = bw[0].tile([128, 512], f16, name=f"rkv{ch}{h}", tag="rkv",
                                 bufs=4)
                nc.scalar.copy(rkv[:], kvt[:, 512 * h:512 * (h + 1)])
                rhs_kv[h] = rkv
                pk = pkkq[:, 256 * h:256 * (h + 1)]
                for i in range(2):
                    qk2 = qkh[ct0 + i].rearrange(
                        "p (n t) -> p n t", n=2)[:, :, t0:t0 + C]
                    nc.tensor.matmul(pk, qkh[ct0 + i][:, T + t0:T + t0 + C],
                                     qk2, start=(i == 0), stop=(i == 1))
                qps = qps_t[:, h:h + 1]
                for i in range(2):
                    nc.tensor.matmul(qps, sq_q[(half, ct0 + i)][
                        :, t0 - half * HALF:t0 - half * HALF + C],
                        ones_col, start=(h == 0 and i == 0), stop=(i == 1),
                        skip_group_check=True)
            qbp = bw[0].tile([128, 2], fp32, name=f"qb{ch}", tag="qb", bufs=4)
            nc.scalar.activation(qbp[:], qps_t[:], AF.Copy, scale=QBS)
            # head-paired mask ops ([h0|h1] strided reads of pkkq)
            pkk2 = pkkq.rearrange("p (h c) -> p h c", h=2)[:, :, 128:256]
            pkq2 = pkkq.rearrange("p (h c) -> p h c", h=2)[:, :, 0:128]
            B2 = Bp.rearrange("p (h c) -> p h c", h=2)
            A2_ = Ap.rearrange("p (h c) -> p h c", h=2)
            M2 = m_su2.rearrange("p (h c) -> p h c", h=2)
            nc.vector.tensor_mul(B2, pkk2, M2)
            nc.vector.tensor_mul(A2_, pkk2,
                                 m_sl2.rearrange("p (h c) -> p h c", h=2))
            for h in range(2):
                hs = slice(128 * h, 128 * (h + 1))
                nc.vector.tensor_sub(R0p[:, hs], ident, Bp[:, hs])
            Patp = bw[0].tile([128, 256], f16, name=f"Pat{ch}", tag="Pat",
                              bufs=4)
            nc.vector.tensor_mul(Patp.rearrange("p (h c) -> p h c", h=2),
                                 pkq2, m_tri2.rearrange("p (h c) -> p h c", h=2))
            Pat = [Patp[:, 0:128], Patp[:, 128:256]]
            st.update(rhs_kv=rhs_kv, Pat=Pat, qb=[qbp[:, 0:1], qbp[:, 1:2]],
                      rp=rp, RT=R0p, Pm=Bp, PTm=Ap)

        def st_rlvl(ch, lvl):
            st = ST[ch]
            rp, RT, Pm, PTm = st["rp"], st["RT"], st["Pm"], st["PTm"]
            for h in range(2):
                hs = slice(128 * h, 128 * (h + 1))
                if lvl < NLVL - 1:
                    nc.tensor.matmul(rp[:, hs], PTm[:, hs], Pm[:, hs],
                                     start=True, stop=True,
                                     skip_group_check=True)
                nc.tensor.matmul(rp[:, 256 + 128 * h:256 + 128 * (h + 1)],
                                 Pm[:, hs], PTm[:, hs], start=True,
                                 stop=True, skip_group_check=True)
            PTn = bw[0].tile([128, 256], f16, name=f"ptn{ch}{lvl}", tag="PT",
                             bufs=4)
            nc.vector.tensor_copy(PTn[:], rp[:, 256:512])
            if lvl < NLVL - 1:
                Pn = bw[0].tile([128, 256], f16, name=f"pn{ch}{lvl}", tag="P",
                                bufs=4)
                nc.scalar.copy(Pn[:], rp[:, 0:256])
            else:
                Pn = None
            for h in range(2):
                hs = slice(128 * h, 128 * (h + 1))
                nc.tensor.matmul(rp[:, hs], PTn[:, hs], RT[:, hs],
                                 start=True, stop=True, skip_group_check=True)
            RTn = bw[0].tile([128, 256], f16, name=f"rt{ch}{lvl}", tag="Rp",
                             bufs=4)
            nc.vector.tensor_add(RTn[:], RT[:], rp[:, 0:256])
            st.update(RT=RTn, Pm=Pn, PTm=PTn)

        def st_zw(ch):
            st = ST[ch]
            RT, rhs_kv = st["RT"], st["rhs_kv"]
            zwp = bw[0].tile([128, 1024], f16, name=f"zwp{ch}", tag="zw")
            wtp = tps.tile([128, 512], f16, name=f"wtp{ch}", tag="tps")
            for h in range(2):
                zw = bigps.tile([128, 512], fp32, name=f"zw{ch}{h}", tag="big")
                nc.tensor.matmul(zw[:], RT[:, 128 * h:128 * (h + 1)],
                                 rhs_kv[h][:], start=True, stop=True)
                if h == 0:
                    nc.scalar.copy(zwp[:, 0:512], zw[:])
                else:
                    nc.scalar.copy(zwp[:, 512:1024], zw[:])
            for h in range(2):
                for i in range(2):
                    nc.tensor.transpose(
                        wtp[:, 256 * h + 128 * i:256 * h + 128 * (i + 1)],
                        zwp[:, 512 * h + 256 + 128 * i:
                            512 * h + 256 + 128 * (i + 1)],
                        ident)
            wts = bw[0].tile([128, 512], f16, name=f"wts{ch}", tag="wt")
            nc.scalar.copy(wts[:], wtp[:])
            st.update(zwp=zwp, wts=wts)

        def st_chain1(ch):
            st = ST[ch]
            zwp, wts = st["zwp"], st["wts"]
            s_prev = [s_sb[0], s_sb[1]]
            up = bw[0].tile([128, 512], f16, name=f"up{ch}", tag="u", bufs=4)
            zsel = zwp.rearrange("p (n c) -> p n c", n=4)[:, 0::2, :]
            if ch == 0:
                nc.vector.tensor_copy(
                    up.rearrange("p (n c) -> p n c", n=2), zsel)
            else:
                pks_t = kps.tile([128, 512], fp32, name=f"pks{ch}", tag="kps")
                for h in range(2):
                    pks = pks_t[:, 256 * h:256 * (h + 1)]
                    for i in range(2):
                        nc.tensor.matmul(
                            pks,
                            wts[:, 256 * h + 128 * i:256 * h + 128 * (i + 1)],
                            s_prev[h][:, i * 256:(i + 1) * 256],
                            start=(i == 0), stop=(i == 1))
                nc.vector.tensor_sub(
                    up.rearrange("p (n c) -> p n c", n=2), zsel, pks_t[:])
            st.update(up=up, s_prev=s_prev)

        def st_chain2(ch):
            st = ST[ch]
            up, s_prev, rhs_kv = st["up"], st["s_prev"], st["rhs_kv"]
            for h in range(2):
                ksu = kps.tile([128, 512], fp32, name=f"ksu{ch}{h}", tag="kps")
                for i in range(2):
                    # start once per bank: start=True marks the WHOLE 2KB
                    # bank pending-zero; i=1's start=False write overwrites
                    # its still-pending half (init semantics).
                    nc.tensor.matmul(
                        ksu[:, i * 256:(i + 1) * 256],
                        rhs_kv[h][:, 256 + 128 * i:256 + 128 * (i + 1)],
                        up[:, 256 * h:256 * (h + 1)],
                        start=(i == 0), stop=True, skip_group_check=True)
                s_n = bw[0].tile([128, 512], f16, name=f"ssb{ch}{h}",
                                 tag="ssb", bufs=4)
                if ch == 0:
                    if h == 0:
                        nc.vector.tensor_copy(s_n[:], ksu[:])
                    else:
                        nc.scalar.copy(s_n[:], ksu[:])
                else:
                    nc.vector.tensor_add(s_n[:], s_prev[h][:], ksu[:])
                s_sb[h] = s_n

        def st_o1(ch):
            st = ST[ch]
            t0 = ch * C
            up, s_prev, Pat, qb = st["up"], st["s_prev"], st["Pat"], st["qb"]
            po_t = rps.tile([128, 512], fp32, name=f"po{ch}", tag="rps")
            onrm = bw[0].tile([128, 512], f16, name=f"onrm{ch}", tag="onrm")
            for h in range(2):
                ct0 = 2 * h
                po = po_t[:, 256 * h:256 * (h + 1)]
                if ch == 0:
                    nc.tensor.matmul(po, Pat[h],
                                     up[:, 256 * h:256 * (h + 1)],
                                     start=(h == 0), stop=True,
                                     skip_group_check=True)
                else:
                    for i in range(2):
                        nc.tensor.matmul(po, qkh[ct0 + i][:, t0:t0 + C],
                                         s_prev[h][:, i * 256:(i + 1) * 256],
                                         start=(h == 0 and i == 0), stop=False,
                                         skip_group_check=True)
                    nc.tensor.matmul(po, Pat[h],
                                     up[:, 256 * h:256 * (h + 1)],
                                     start=False, stop=True,
                                     skip_group_check=True)
                osq = bw[0].tile([128, 256], f16, name=f"osq{ch}{h}",
                                 tag="osq")
                ossq = bw[0].tile([128, 1], fp32, name=f"ossq{ch}{h}",
                                  tag="ossq", bufs=4)
                nc.scalar.activation(osq[:], po, AF.Square, accum_out=ossq[:])
                orsq = bw[0].tile([128, 1], fp32, name=f"orsq{ch}{h}",
                                  tag="orsq", bufs=4)
                nc.scalar.activation(orsq[:], ossq[:], AF.Sqrt,
                                     bias=qb[h], scale=1.0 / DH)
                nc.vector.reciprocal(orsq[:], orsq[:])
                nc.vector.tensor_scalar_mul(onrm[:, 256 * h:256 * (h + 1)],
                                            po, orsq[:])
            st.update(onrm=onrm)

        def st_o2c(ch):
            st = ST[ch]
            t0 = ch * C
            onrm = st["onrm"]
            otp = tps.tile([128, 512], f16, name=f"otp{ch}", tag="tps")
            for h in range(2):
                for i in range(2):
                    nc.tensor.transpose(
                        otp[:, 256 * h + 128 * i:256 * h + 128 * (i + 1)],
                        onrm[:, 256 * h + i * 128:256 * h + (i + 1) * 128],
                        ident)
                dstp = oTp[h].rearrange("p (n t) -> p n t",
                                        n=2)[:, :, t0:t0 + C]
                srcp = otp[:, 256 * h:256 * (h + 1)].rearrange(
                    "p (n t) -> p n t", n=2)
                nc.scalar.activation(dstp, srcp, AF.Copy)
            for hf in range(2):
                pf = bigps.tile([128, 512], fp32, name=f"pf{ch}{hf}",
                                tag="big")
                for ct in range(4):
                    h, i = divmod(ct, 2)
                    nc.tensor.matmul(
                        pf[:], oTp[h][:, i * T + t0:i * T + t0 + C],
                        wo_s[ct][:, hf * 512:(hf + 1) * 512],
                        start=(ct == 0), stop=(ct == 3))
                of = ofp.tile([128, 512], f16, name=f"of{ch}{hf}", tag="of")
                nc.scalar.copy(of[:], pf[:])
                nc.sync.dma_start(out_t[ch][:, hf * 512:(hf + 1) * 512], of[:])
            del ST[ch]

        # ============ top-level emission order ============
        emit_proj_block(0, 0)
        emit_proj_block(0, 1)
        for ti in range(3):
            for ct in range(4):
                emit_conv_taps(0, ti, ct)
                emit_silu(0, ti, ct)
        emit_norms(0)
        emit_proj_block(1, 0)
        emit_proj_block(1, 1)
        # projection inputs are dead now; reuse their SBUF for phase B work
        xwp.release()
        bw[0] = tc.alloc_tile_pool(name="bwork", bufs=3)
        for h in range(2):
            t_ = bw[0].tile([128, 512], f16, name=f"ssb{h}_init", tag="ssb",
                            bufs=4)
            nc.vector.memset(t_[:], 0.0)
            s_sb[h] = t_

        def _silus_norms():
            for ti in range(3):
                for ct in range(4):
                    emit_silu(1, ti, ct)
            emit_norms(1)

        a1 = [
            lambda: [emit_conv_taps(1, 0, ct) for ct in range(4)],
            lambda: [emit_conv_taps(1, 1, ct) for ct in range(4)],
            lambda: [emit_conv_taps(1, 2, ct) for ct in range(4)],
            _silus_norms,
        ]

        for it in range(NCHUNK // 2 + 1):
            c0, c1 = 2 * it, 2 * it + 1
            p0, p1 = c0 - 2, c1 - 2
            pre = c0 < NCHUNK
            if pre:
                st_pre(c0)
                st_pre(c1)
                st_rlvl(c0, 0)
                st_rlvl(c1, 0)
            if p0 >= 0:
                st_chain1(p0)
            if pre:
                st_rlvl(c0, 1)
                st_rlvl(c1, 1)
            if p0 >= 0:
                st_chain2(p0)
                st_chain1(p1)
            if pre:
                st_rlvl(c0, 2)
                st_rlvl(c1, 2)
            if p0 >= 0:
                st_chain2(p1)
                st_o1(p0)
            if pre:
                st_rlvl(c0, 3)
                st_rlvl(c1, 3)
            if p0 >= 0:
                st_o1(p1)
                st_o2c(p0)
                st_o2c(p1)
            if pre:
                st_zw(c0)
                st_zw(c1)
            if it < len(a1):
                a1[it]()
        bw[0].release()


LP_NP = np.float16


def _make_consts():
    ii = np.arange(128)
    ident = np.eye(128, dtype=np.float32)
    m_su = (ii[:, None] < ii[None, :]).astype(np.float32)
    m_sl = (ii[:, None] > ii[None, :]).astype(np.float32)
    m_R0 = ident - m_su
    m_triuI = (ii[:, None] <= ii[None, :]).astype(np.float32)
    return np.concatenate([ident, m_su, m_su, m_sl, m_sl, m_R0, m_R0,
                           m_triuI, m_triuI], axis=1).astype(LP_NP)


def _get_compiled():
    if "nc" not in _CACHE:
        _CACHE["nc"] = _build_bass()
    return _CACHE["nc"]


def kernel(hidden_states, Wq, Wk, Wv, conv_wq, conv_wk, conv_wv, onorm_w, Wo):
    from concourse.bass_utils import run_bass_kernel_spmd

    hidden_states = np.asarray(hidden_states, np.float32)
    Wq = np.asarray(Wq, np.float32)
    Wk = np.asarray(Wk, np.float32)
    Wv = np.asarray(Wv, np.float32)
    Wo = np.asarray(Wo, np.float32)
    conv_wq = np.asarray(conv_wq, np.float32)
    conv_wk = np.asarray(conv_wk, np.float32)
    conv_wv = np.asarray(conv_wv, np.float32)
    onorm_w = np.asarray(onorm_w, np.float32)

    consts = _make_consts()
    Wo_eff = (Wo * np.tile(onorm_w, H)[:, None]).astype(LP_NP)

    in_maps = []
    for core in range(NCORES):
        b, g = divmod(core, 2)
        cols = slice(CG * g, CG * (g + 1))
        cwf = np.concatenate([conv_wq[cols], conv_wk[cols], conv_wv[cols]],
                             axis=1)
        in_maps.append({
            "xT": np.ascontiguousarray(hidden_states[b].T).astype(LP_NP),
            "wq": np.ascontiguousarray(Wq[:, cols]).astype(LP_NP),
            "wk": np.ascontiguousarray(Wk[:, cols]).astype(LP_NP),
            "wv": np.ascontiguousarray(Wv[:, cols]).astype(LP_NP),
            "wo": np.ascontiguousarray(Wo_eff[cols, :]),
            "cw": np.ascontiguousarray(cwf.reshape(4, 128, 3 * CONV_K)),
            "consts": consts,
        })

    nc = _get_compiled()
    res = run_bass_kernel_spmd(nc, in_maps, core_ids=list(range(NCORES)),
                               **_CACHE.get("run_kwargs", {}))
    _CACHE["last_results"] = res
    out = np.zeros((B, T, D), np.float32)
    for core in range(NCORES):
        out[core // 2] += res.results[core]["out"].astype(np.float32)
    return out
